# revision 23
# baseline (speedup 1.0000x reference)
"""AttnBlock (GroupNorm + single-head self-attention + residual) on 8 TRN2 cores.

Sharding: data-parallel over (batch b, query-half h) -> 8 shards. Each core
receives the full [C, N] image of its batch (columns rolled so that its own
query half always occupies columns 0:NQ), computes GroupNorm stats + K/V over
the whole image, Q over its half, and a flash-style attention in which scores
are produced directly transposed (S^T = K^T.T @ Q^T tiles) so softmax
normalization is done via a ones-vector matmul and no PE transposes of P are
needed.

All large matmuls (projections, S^T, PV, denominator, out-proj) run as fp8e4
DoubleRow matmuls: lhsT [128, 2, M] / rhs [128, 2, N] contract 256 deep in a
single instruction at ~2x bf16 FLOP rate. Weights are scaled x8 before the
fp8 cast (their entries are ~N(0, 1/16) and would hit e4m3 subnormals); the
scale is removed in the PSUM->SBUF cast. The softmax exp is shifted by -2
(exp(s/16 - 2)) so P fits e4m3's +-240 range; numerator and denominator share
the shift so the ratio is unchanged. exp runs on ACT in 2-key-tile batches
([128,1024] over a 2-bank PSUM tile) to amortize the per-call overhead, and
the denominator is a DoubleRow ones-matmul on the PE (accumulated in a
[1,512] PSUM bank), keeping the DVE free for casts and the epilogue.
"""

import os
import sys

import numpy as np

for _p in ("/opt/trn_rl_repo", "/root/.axon_site/_ro/trn_rl_repo"):
    if os.path.isdir(_p) and _p not in sys.path:
        sys.path.insert(0, _p)

import concourse.bass as bass  # noqa: E402
import concourse.tile as tile  # noqa: E402
from concourse import bacc, mybir  # noqa: E402
from concourse.masks import make_identity  # noqa: E402

# The agent image's antenv lacks axon_hooks; if BASS_TRACE is set in the
# environment, run_bass_kernel_spmd would crash importing it. Provide a stub
# (profiling degrades gracefully to "hook isn't registered").
try:
    import antenv.axon_hooks  # noqa: F401
except ImportError:
    import types as _types

    _m = _types.ModuleType("antenv.axon_hooks")
    _h = [None]
    _m.set_axon_ntff_profile_hook = lambda h: _h.__setitem__(0, h)
    _m.get_axon_ntff_profile_hook = lambda: _h[0]
    sys.modules["antenv.axon_hooks"] = _m

B, C, H, W = 4, 256, 64, 64
N = H * W  # 4096 pixels
NQ = N // 2  # 2048 queries per core
G = 32  # groups
CPG = C // G  # 8 channels per group
EPS = 1e-5
NCORES = 8
SCALE = float(C) ** -0.5  # 0.0625
ESHIFT = -3.0  # exp(s*SCALE + ESHIFT): data max logit 7.95 -> P <= ~141 < 240
WS = 8.0  # weight fp8 pre-scale (entries ~N(0,1/16) need lifting)

F32 = mybir.dt.float32
BF16 = mybir.dt.bfloat16
FP8 = mybir.dt.float8e4

QB = 512  # query block (free dim of S^T / PV matmuls)
NQB = NQ // QB  # 4 query blocks
NKT = N // 128  # 32 key tiles
NKP = NKT // 2  # 16 key-tile pairs (DoubleRow granularity)
NNB = N // QB  # 8 pixel blocks for K/V projections
P = 128

DEBUG = bool(int(os.environ.get("KDEBUG", "0")))

Act = mybir.ActivationFunctionType
Alu = mybir.AluOpType
Axis = mybir.AxisListType
DR = mybir.MatmulPerfMode.DoubleRow

_NC = None
LAST_RESULTS = None


def _body(tc, d):
    nc = tc.nc
    x_d = d["x"]
    out_d = d["out"]

    const = tc.alloc_tile_pool(name="const", bufs=1)
    stage = tc.alloc_tile_pool(name="stage", bufs=2)
    small = tc.alloc_tile_pool(name="small", bufs=1)
    pblk = tc.alloc_tile_pool(name="pblk", bufs=2)
    work = tc.alloc_tile_pool(name="work", bufs=2)
    # PSUM budget (8 banks): S-pair tiles 2x[P,1024] = 4, aps 2x[P,512] = 2,
    # dps [1,512] = 1, po [P,512] = 1.
    ps = tc.alloc_tile_pool(name="ps", bufs=2, space="PSUM")
    ps_acc = tc.alloc_tile_pool(name="ps_acc", bufs=2, space="PSUM")
    ps_d = tc.alloc_tile_pool(name="ps_d", bufs=1, space="PSUM")
    ps_o = tc.alloc_tile_pool(name="ps_o", bufs=1, space="PSUM")

    # ---- x in SBUF first: bf16 [128, 2(ch), 4096] (residual + stats) and
    # fp8 [128, 2, 4096] (matmul operand), both cast on host; chunked so
    # bn_stats overlaps the transfer ----
    x_sb = const.tile([P, 2, N], BF16)
    x8 = const.tile([P, 2, N], FP8)
    x_src = x_d.ap().rearrange("(h p) n -> p h n", p=P)
    x8_src = d["x8"].ap().rearrange("(h p) n -> p h n", p=P)

    # PE warm-up: keep the HAM activity monitor busy during the DMA/stats
    # window so projections and attention run at full clock from the start.
    # Memsets go FIRST so the warm matmuls aren't queued behind the bn_stats
    # chain in DVE program order (PE would idle for the whole DMA phase).
    wu_w = const.tile([P, P], BF16)
    nc.vector.memset(wu_w, 0.0)
    wu_x = const.tile([P, 2 * P], BF16)
    nc.vector.memset(wu_x, 0.0)
    wu_ps = ps.tile([P, QB], F32, name="wu_ps", tag="mm")

    def warm(n):
        for _ in range(n):
            nc.tensor.matmul(
                wu_ps[:, 0:2 * P], lhsT=wu_w, rhs=wu_x, start=True, stop=True
            )

    warm(2)

    # group-reduce/broadcast masks (host-built): M1[p,g]=1 iff p//8==g,
    # M2 = M1^T. They turn the GroupNorm channel->group reduction and the
    # group->channel broadcast into two tiny PE matmuls (no transposes).
    m1_sb = const.tile([P, G // 2], F32)
    nc.sync.dma_start(out=m1_sb, in_=d["m1"][:, :])
    m2_sb = const.tile([G // 2, P], F32)
    nc.sync.dma_start(out=m2_sb, in_=d["m2"][:, :])
    # packed biases/affine: rows (bq,bk,bv,bo,gamma,beta) -> per-channel cols
    bcols = const.tile([P, 2, 6], F32)
    nc.gpsimd.dma_start(
        out=bcols, in_=d["bpack"].ap().rearrange("(h p) s -> p h s", p=P)
    )
    # all four weight matrices in one DMA: [P, w, h, co]
    wstg = stage.tile([P, 4, 2, C], F32, name="wstg", tag="wstg")
    nc.gpsimd.dma_start(
        out=wstg, in_=d["wpack"].ap().rearrange("(w h p) co -> p w h co", p=P, h=2)
    )

    # ---- x DMA in 4 dual-plane column chunks; GroupNorm stats split across
    # DVE (bn_stats: ch0 + first half of ch1) and ACT (sum / sum-sq with the
    # free accumulator: second half of ch1) so neither engine paces the head.
    bn_st0 = small.tile([P, NNB, 6], F32, name="bn_st0")
    bn_st1a = small.tile([P, 4, 6], F32, name="bn_st1a")
    sx1 = small.tile([P, 4], F32, name="sx1")
    sq1 = small.tile([P, 4], F32, name="sq1")
    scr = small.tile([P, QB], BF16, name="scr")
    for c in range(4):
        sl = (slice(None), slice(None), slice(c * 2 * QB, (c + 1) * 2 * QB))
        nc.sync.dma_start(out=x_sb[sl], in_=x_src[sl])
        nc.gpsimd.dma_start(out=x8[sl], in_=x8_src[sl])
        for k in range(2):
            j = 2 * c + k
            cols = slice(j * QB, (j + 1) * QB)
            nc.vector.bn_stats(out=bn_st0[:, j, :], in_=x_sb[:, 0, cols])
            if j < 4:
                nc.vector.bn_stats(out=bn_st1a[:, j, :], in_=x_sb[:, 1, cols])
            else:
                nc.scalar.activation(
                    scr, x_sb[:, 1, cols], Act.Identity,
                    accum_out=sx1[:, j - 4:j - 3],
                )
                nc.scalar.activation(
                    scr, x_sb[:, 1, cols], Act.Square,
                    accum_out=sq1[:, j - 4:j - 3],
                )
        warm(10)

    # ---- constants ----
    one11 = const.tile([1, 1], F32)
    nc.vector.memset(one11, 1.0)
    # padded to 16B so the DoubleRow k-tile stride meets walrus' step%16==0.
    # Value 1.0 balances the at_sb 1/8 down-scale and the x8 wo lift:
    # po*den_r = (8/8)*wo@at_un / den = wo@at_un/den.
    ones8_pad = const.tile([P, 2, 16], FP8)
    nc.vector.memset(ones8_pad, 1.0)
    ones8 = ones8_pad[:, :, 0:1]
    eps16 = const.tile([G // 2, 1], F32)
    nc.vector.memset(eps16, EPS)
    esh_col = const.tile([P, 1], F32)
    nc.vector.memset(esh_col, ESHIFT)
    # preload the ACT ln/exp tables (one set serves the GroupNorm rstd
    # e^{-0.5 ln(var+eps)} trick AND the attention exp)
    warm11 = small.tile([1, 1], F32)
    nc.scalar.activation(warm11, one11, Act.Ln, scale=1.0)
    warm12 = small.tile([1, 1], F32)
    nc.scalar.activation(warm12, one11, Act.Exp, scale=1.0)

    # ---- GroupNorm dance: per-channel (mean, E[x^2]) -> group reduce via
    # mask matmul -> rstd -> broadcast back via mask matmul -> a/b columns.
    mvex = small.tile([P, 2, 2], F32, name="mvex")  # (mean, ex2) per ch-half
    mv0 = small.tile([P, 2], F32, name="mv0")
    nc.vector.bn_aggr(out=mv0, in_=bn_st0)
    mv1a = small.tile([P, 2], F32, name="mv1a")
    nc.vector.bn_aggr(out=mv1a, in_=bn_st1a)
    tcol = small.tile([P, 6], F32, name="tcol")
    # ch0: mean, ex2 = var + mean^2
    nc.vector.tensor_copy(out=mvex[:, 0, 0:1], in_=mv0[:, 0:1])
    nc.vector.tensor_mul(tcol[:, 0:1], mv0[:, 0:1], mv0[:, 0:1])
    nc.vector.tensor_add(mvex[:, 0, 1:2], tcol[:, 0:1], mv0[:, 1:2])
    # ch1: combine bn_aggr over cols 0:2048 with ACT sums over 2048:4096
    nc.vector.tensor_reduce(out=tcol[:, 1:2], in_=sx1, axis=Axis.X, op=Alu.add)
    nc.vector.tensor_reduce(out=tcol[:, 2:3], in_=sq1, axis=Axis.X, op=Alu.add)
    nc.vector.tensor_scalar_mul(tcol[:, 3:4], tcol[:, 1:2], 1.0 / N)
    nc.vector.scalar_tensor_tensor(
        out=mvex[:, 1, 0:1], in0=mv1a[:, 0:1], scalar=0.5, in1=tcol[:, 3:4],
        op0=Alu.mult, op1=Alu.add,
    )
    nc.vector.tensor_mul(tcol[:, 4:5], mv1a[:, 0:1], mv1a[:, 0:1])
    nc.vector.tensor_add(tcol[:, 4:5], tcol[:, 4:5], mv1a[:, 1:2])
    nc.vector.tensor_scalar_mul(tcol[:, 5:6], tcol[:, 2:3], 1.0 / N)
    nc.vector.scalar_tensor_tensor(
        out=mvex[:, 1, 1:2], in0=tcol[:, 4:5], scalar=0.5, in1=tcol[:, 5:6],
        op0=Alu.mult, op1=Alu.add,
    )
    # group sums over partitions: [16 groups, (mean0, ex20, mean1, ex21)]
    gsum = ps_o.tile([G // 2, 4], F32, name="gsum", tag="po")
    nc.tensor.matmul(
        gsum, lhsT=m1_sb, rhs=mvex.rearrange("p c k -> p (c k)"),
        start=True, stop=True,
    )
    warm(4)
    vals = small.tile([G // 2, 4], F32, name="vals")  # (rstd0, rstd1, m0, m1)
    gtmp = small.tile([G // 2, 4], F32, name="gtmp")
    gview = gsum.rearrange("g (c k) -> g c k", k=2)
    nc.vector.tensor_scalar_mul(
        vals.rearrange("g (c k) -> g c k", k=2)[:, 1, :], gview[:, :, 0],
        1.0 / CPG,
    )
    nc.vector.tensor_scalar_mul(gtmp[:, 0:2], gview[:, :, 1], 1.0 / CPG)
    nc.vector.tensor_mul(gtmp[:, 2:4], vals[:, 2:4], vals[:, 2:4])
    nc.vector.tensor_sub(gtmp[:, 0:2], gtmp[:, 0:2], gtmp[:, 2:4])
    # rstd = exp(-0.5 * ln(var + eps)) - same ACT table set as attention exp
    nc.scalar.activation(gtmp[:, 0:2], gtmp[:, 0:2], Act.Ln, bias=eps16)
    nc.scalar.activation(vals[:, 0:2], gtmp[:, 0:2], Act.Exp, scale=-0.5)
    # broadcast back to channels: [128, (rstd0, rstd1, m0, m1)]
    bc = ps_acc.tile([P, 4], F32, name="bc", tag="acc")
    nc.tensor.matmul(bc, lhsT=m2_sb, rhs=vals, start=True, stop=True)
    warm(4)
    # a = gamma * rstd; b = beta - mean * a; a8 = 8a (per channel cols)
    cols8 = small.tile([P, 8], F32, name="cols8")
    a_cols = [cols8[:, 0:1], cols8[:, 1:2]]
    b_cols = [cols8[:, 2:3], cols8[:, 3:4]]
    a8_cols = [cols8[:, 4:5], cols8[:, 5:6]]
    for ch in range(2):
        nc.vector.tensor_mul(a_cols[ch], bcols[:, ch, 4:5], bc[:, ch:ch + 1])
        nc.vector.tensor_mul(cols8[:, 6 + ch:7 + ch], bc[:, 2 + ch:3 + ch],
                             a_cols[ch])
        nc.vector.tensor_sub(b_cols[ch], bcols[:, ch, 5:6],
                             cols8[:, 6 + ch:7 + ch])
        nc.vector.tensor_scalar_mul(a8_cols[ch], a_cols[ch], WS)

    # scale wq/wk/wv rows by 8*a (per input channel) straight from the f32
    # staging into fp8 tiles. The x8 lift keeps the fp8 entries
    # (~N(0, a/16)) out of e4m3 subnormal range; the PSUM->SBUF casts
    # divide it back out.
    w_s = {}
    for wi, wname in ((0, "wqt"), (1, "wkt"), (2, "wvt")):
        ws = const.tile([P, 2, C], FP8, name=f"{wname}_s")
        for ci in range(2):
            nc.vector.tensor_scalar_mul(
                ws[:, ci, :], wstg[:, wi, ci, :], a8_cols[ci]
            )
        w_s[wname] = ws
    # wo8 = 8*wo in fp8 (no GroupNorm folding on the out-proj)
    wo8 = const.tile([P, 2, C], FP8)
    for ch in range(2):
        nc.scalar.mul(wo8[:, ch, :], wstg[:, 3, ch, :], WS)

    # projection bias columns: be = W b + bias (f32 matvecs off the staged
    # weights; psum ring across three pools so the chain pipelines)
    _mvi = [0]
    mv_pools = [(ps_o, "po"), (ps_acc, "acc"), (ps, "mm")]

    def matvec_bias(wi, rhs_cols, bias_idx, out_name):
        outs = []
        for co in range(2):
            pool, tag = mv_pools[_mvi[0] % 3]
            _mvi[0] += 1
            pe = pool.tile([P, 1], F32, name="pe_mv", tag=tag)
            for ci in range(2):
                nc.tensor.matmul(
                    pe, lhsT=wstg[:, wi, ci, co * P:(co + 1) * P],
                    rhs=rhs_cols[ci], start=(ci == 0), stop=(ci == 1),
                )
            t = small.tile([P, 1], F32, name=f"{out_name}_{co}")
            nc.scalar.activation(
                t, pe, Act.Identity, bias=bcols[:, co, bias_idx:bias_idx + 1],
                scale=1.0,
            )
            outs.append(t)
        return outs

    be_q = matvec_bias(0, b_cols, 0, "be_q")
    be_k = matvec_bias(1, b_cols, 1, "be_k")
    vbv = matvec_bias(2, b_cols, 2, "vbv")
    bo_eff = matvec_bias(3, vbv, 3, "bo_eff")
    warm(4)

    # ---- projections (all DoubleRow fp8, contraction over full C=256) ----
    # K^T [C, N] fp8: psum[co,nb] = sum_ci wkt8[ci,co].T @ x8[ci, nb] (x8)
    # and q,k = psum/8 + bias. Casts alternate ACT / DVE (ACT is idle until
    # the attention loop's exp stream starts).
    k_sb = const.tile([P, 2, N], FP8)
    q_sb = const.tile([P, 2, NQ], FP8)
    IWS = 1.0 / WS
    # casts rotate over three engines (ACT/DVE/Pool) and the psum tiles over
    # three pools (5 in-flight banks) so the MM stream never waits on a cast;
    # two-engine casting was the pacer that kept HAM throttled at K=4.
    proj_ps = [(ps, "mm"), (ps_acc, "acc"), (ps, "mm"), (ps_o, "po"),
               (ps_acc, "acc")]
    _pi = [0]

    def proj_tile():
        pool, tag = proj_ps[_pi[0] % len(proj_ps)]
        _pi[0] += 1
        return pool.tile([P, QB], F32, name="pp", tag=tag)

    _ci = [0]

    def proj_cast(out, in_, bias):
        # only ACT/DVE can read PSUM; the 5-bank psum ring above keeps the
        # MM stream ~2.5 tiles ahead so the 2-engine cast pace (~325ns/tile)
        # doesn't stall the PE.
        i = _ci[0] % 2
        _ci[0] += 1
        if i == 0:
            nc.scalar.activation(out, in_, Act.Identity, bias=bias, scale=IWS)
        else:
            nc.vector.tensor_scalar(
                out=out, in0=in_, scalar1=IWS, scalar2=bias, op0=Alu.mult,
                op1=Alu.add,
            )

    for nb in range(NNB):
        for co in range(2):
            if nb < NQB:
                pq = proj_tile()
                nc.tensor.matmul(
                    pq, lhsT=w_s["wqt"][:, :, co * P:(co + 1) * P],
                    rhs=x8[:, :, nb * QB:(nb + 1) * QB],
                    start=True, stop=True, perf_mode=DR,
                )
                proj_cast(q_sb[:, co, nb * QB:(nb + 1) * QB], pq, be_q[co])
            pk = proj_tile()
            nc.tensor.matmul(
                pk, lhsT=w_s["wkt"][:, :, co * P:(co + 1) * P],
                rhs=x8[:, :, nb * QB:(nb + 1) * QB],
                start=True, stop=True, perf_mode=DR,
            )
            proj_cast(k_sb[:, co, nb * QB:(nb + 1) * QB], pk, be_k[co])

    # V [N, C] fp8 (bias folded into bo_eff): psum[nt] = x8_tile.T @ wvt8
    v_sb = const.tile([P, NKT, C], FP8)
    v_flat = v_sb.rearrange("p k c -> p (k c)")
    zero_col = const.tile([P, 1], F32)
    nc.vector.memset(zero_col, 0.0)
    for nt in range(0, NKT, 2):
        pv = proj_tile()
        for n2 in range(2):
            nc.tensor.matmul(
                pv[:, n2 * C:(n2 + 1) * C],
                lhsT=x8[:, :, (nt + n2) * P:(nt + n2 + 1) * P],
                rhs=w_s["wvt"][:, :, :],
                start=True, stop=True, perf_mode=DR,
            )
        proj_cast(v_flat[:, nt * C:(nt + 2) * C], pv, zero_col)

    # ---- attention, per query block; DoubleRow over key-tile pairs with a
    # batched exp (one ACT call per pair reading a 2-bank PSUM tile) and the
    # denominator as a DoubleRow ones-matmul on the PE. The softmax division
    # is commuted through the out-projection: out = (wo8 @ PV) * (1/(8*den))
    # + bo_eff + x, deferred one qb so nothing waits on the reciprocal.
    def epilogue_a(qb, dps, aps, at_sb, p_sb_dbg=None):
        # casts first: they release the PV accumulator banks immediately.
        # 1/8 keeps the heavy-tailed PV numerator inside fp8's +-240.
        nc.vector.tensor_scalar_mul(at_sb[:, 0, :], aps[0], 1.0 / 8.0)
        nc.vector.tensor_scalar_mul(at_sb[:, 1, :], aps[1], 1.0 / 8.0)
        den_r = work.tile([1, QB], F32, name="den_r", tag="den_r")
        nc.vector.reciprocal_approx_fast(out=den_r, in_=dps)
        den_b = work.tile([P, QB], F32, name="den_b", tag="den_b", bufs=2)
        nc.gpsimd.partition_broadcast(den_b, den_r)
        if DEBUG:
            nc.sync.dma_start(
                out=d["dbg_denr"][:, qb * QB:(qb + 1) * QB], in_=den_r
            )
            nc.sync.dma_start(
                out=d["dbg_at"][:, qb * 2 * QB:(qb + 1) * 2 * QB],
                in_=at_sb.rearrange("p h n -> p (h n)"),
            )
            if qb == 0:
                nc.sync.dma_start(
                    out=d["dbg_p"][:, :], in_=p_sb_dbg.rearrange("p k n -> p (k n)")
                )
        return den_b

    def epilogue_co(qb, at_sb, den_b, co, po_pool=None, po_tag="po"):
        po_pool = po_pool or ps_o
        po = po_pool.tile([P, QB], F32, name="po", tag=po_tag)
        nc.tensor.matmul(
            po, lhsT=wo8[:, :, co * P:(co + 1) * P], rhs=at_sb,
            start=True, stop=True, perf_mode=DR,
        )
        t1 = work.tile([P, QB], F32, name="t1", tag="t1")
        nc.vector.tensor_mul(t1, po, den_b)
        res = work.tile([P, QB], F32, name="res", tag="res", bufs=4)
        nc.vector.scalar_tensor_tensor(
            out=res, in0=t1, scalar=bo_eff[co],
            in1=x_sb[:, co, qb * QB:(qb + 1) * QB], op0=Alu.add, op1=Alu.add,
        )
        nc.sync.dma_start(
            out=out_d[co * P:(co + 1) * P, qb * QB:(qb + 1) * QB], in_=res
        )

    if DEBUG:
        nc.sync.dma_start(out=d["dbg_k"][:, :], in_=k_sb.rearrange("p h n -> p (h n)"))
        nc.sync.dma_start(out=d["dbg_q"][:, :], in_=q_sb.rearrange("p h n -> p (h n)"))
        nc.sync.dma_start(out=d["dbg_v"][:, :], in_=v_flat)

    pending = None
    pend_den = None
    for qb in range(NQB):
        p_sb = pblk.tile([P, NKT, QB], FP8, name="p_sb")
        dps = ps_d.tile([1, QB], F32, name="dps")
        aps = [
            ps_acc.tile([P, QB], F32, name="aps", tag="acc") for _ in range(2)
        ]
        at_sb = work.tile([P, 2, QB], FP8, name="at_sb", tag="at_sb", bufs=2)
        for j in range(NKP + 2):
            if j == 1 and pending is not None:
                pend_den = epilogue_a(*pending)
            if j == 2 and pending is not None:
                epilogue_co(pending[0], pending[3], pend_den, 0)
            if j == 3 and pending is not None:
                epilogue_co(pending[0], pending[3], pend_den, 1)
                pending = None
            if j < NKP:
                sps2 = ps.tile([P, 2 * QB], F32, name="sps2", tag="mm")
                for h2 in range(2):
                    kt = 2 * j + h2
                    nc.tensor.matmul(
                        sps2[:, h2 * QB:(h2 + 1) * QB],
                        lhsT=k_sb[:, :, kt * P:(kt + 1) * P],
                        rhs=q_sb[:, :, qb * QB:(qb + 1) * QB],
                        start=True, stop=True, perf_mode=DR,
                    )
                nc.scalar.activation(
                    p_sb[:, 2 * j:2 * j + 2, :], sps2, Act.Exp,
                    bias=esh_col, scale=SCALE,
                )
            if j >= 2:
                pj = j - 2
                pair = p_sb[:, 2 * pj:2 * pj + 2, :]
                for ch in range(2):
                    nc.tensor.matmul(
                        aps[ch],
                        lhsT=v_sb[:, 2 * pj:2 * pj + 2, ch * P:(ch + 1) * P],
                        rhs=pair,
                        start=(pj == 0), stop=(pj == NKP - 1),
                        perf_mode=DR, skip_group_check=True,
                    )
                nc.tensor.matmul(
                    dps, lhsT=ones8, rhs=pair,
                    start=(pj == 0), stop=(pj == NKP - 1),
                    perf_mode=DR, skip_group_check=True,
                )
        pending = (qb, dps, aps, at_sb, p_sb)
    pend_den = epilogue_a(*pending)
    epilogue_co(pending[0], pending[3], pend_den, 0, po_pool=ps, po_tag="mm")
    epilogue_co(pending[0], pending[3], pend_den, 1, po_pool=ps, po_tag="mm")

    for pool in (ps_o, ps_d, ps_acc, ps, work, pblk, small, stage, const):
        pool.release()


def build_program():
    global _NC
    if _NC is not None:
        return _NC
    nc = bacc.Bacc("TRN2", target_bir_lowering=False, debug=False,
                   num_devices=NCORES)
    d = {
        "x": nc.dram_tensor("x", [C, N], BF16, kind="ExternalInput"),
        "x8": nc.dram_tensor("x8", [C, N], FP8, kind="ExternalInput"),
        "wpack": nc.dram_tensor("wpack", [4 * C, C], F32, kind="ExternalInput"),
        "bpack": nc.dram_tensor("bpack", [C, 6], F32, kind="ExternalInput"),
        "m1": nc.dram_tensor("m1", [P, G // 2], F32, kind="ExternalInput"),
        "m2": nc.dram_tensor("m2", [G // 2, P], F32, kind="ExternalInput"),
        "out": nc.dram_tensor("out", [C, NQ], F32, kind="ExternalOutput"),
    }
    if DEBUG:
        d.update({
            "dbg_k": nc.dram_tensor("dbg_k", [P, 2 * N], FP8, kind="ExternalOutput"),
            "dbg_q": nc.dram_tensor("dbg_q", [P, 2 * NQ], FP8, kind="ExternalOutput"),
            "dbg_v": nc.dram_tensor("dbg_v", [P, NKT * C], FP8, kind="ExternalOutput"),
            "dbg_p": nc.dram_tensor("dbg_p", [P, NKT * QB], FP8, kind="ExternalOutput"),
            "dbg_denr": nc.dram_tensor("dbg_denr", [1, NQ], F32, kind="ExternalOutput"),
            "dbg_at": nc.dram_tensor("dbg_at", [P, NQB * 2 * QB], FP8, kind="ExternalOutput"),
        })
    with tile.TileContext(nc) as tc:
        _body(tc, d)
    nc.compile()
    _NC = nc
    return nc


def make_in_maps(x, gamma, beta, wq, bq, wk, bk, wv, bv, wo, bo):
    f32c = lambda a: np.ascontiguousarray(np.asarray(a, dtype=np.float32))
    x = f32c(x)
    wpack = np.concatenate(
        [f32c(np.asarray(w, np.float32).T) for w in (wq, wk, wv, wo)], axis=0
    )
    bpack = np.stack(
        [f32c(v).reshape(C) for v in (bq, bk, bv, bo, gamma, beta)], axis=1
    )
    m1 = np.zeros((P, G // 2), np.float32)
    for g in range(G // 2):
        m1[8 * g:8 * g + 8, g] = 1.0
    base = {
        "wpack": np.ascontiguousarray(wpack),
        "bpack": np.ascontiguousarray(bpack),
        "m1": m1,
        "m2": np.ascontiguousarray(m1.T),
    }
    import ml_dtypes

    in_maps = []
    for core in range(NCORES):
        b, h = divmod(core, 2)
        xb = x[b].reshape(C, N)
        if h:
            xb = np.concatenate([xb[:, NQ:], xb[:, :NQ]], axis=1)
        in_maps.append({
            **base,
            "x": np.ascontiguousarray(xb.astype(ml_dtypes.bfloat16)),
            "x8": np.ascontiguousarray(xb.astype(ml_dtypes.float8_e4m3)),
        })
    return in_maps


def kernel(x, gamma, beta, wq, bq, wk, bk, wv, bv, wo, bo):
    global LAST_RESULTS
    from concourse.bass_utils import run_bass_kernel_spmd

    nc = build_program()
    in_maps = make_in_maps(x, gamma, beta, wq, bq, wk, bk, wv, bv, wo, bo)
    res = run_bass_kernel_spmd(nc, in_maps, core_ids=list(range(NCORES)))
    LAST_RESULTS = res
    out = np.empty((B, C, N), np.float32)
    for core in range(NCORES):
        b, h = divmod(core, 2)
        out[b][:, h * NQ:(h + 1) * NQ] = res.results[core]["out"]
    return out.reshape(B, C, H, W)


# revision 33
# speedup vs baseline: 1.0394x; 1.0394x over previous
"""AttnBlock (GroupNorm + single-head self-attention + residual) on 8 TRN2 cores.

Sharding: data-parallel over (batch b, query-half h) -> 8 shards. Each core
receives the full [C, N] image of its batch (columns rolled so that its own
query half always occupies columns 0:NQ), computes GroupNorm stats + K/V over
the whole image, Q over its half, and a flash-style attention in which scores
are produced directly transposed (S^T = K^T.T @ Q^T tiles) so softmax
normalization is done via a ones-vector matmul and no PE transposes of P are
needed.

All large matmuls (projections, S^T, PV, denominator, out-proj) run as fp8e4
DoubleRow matmuls: lhsT [128, 2, M] / rhs [128, 2, N] contract 256 deep in a
single instruction at ~2x bf16 FLOP rate. Weights are scaled x8 before the
fp8 cast (their entries are ~N(0, 1/16) and would hit e4m3 subnormals); the
scale is removed in the PSUM->SBUF cast. The softmax exp is shifted by -2
(exp(s/16 - 2)) so P fits e4m3's +-240 range; numerator and denominator share
the shift so the ratio is unchanged. exp runs on ACT in 2-key-tile batches
([128,1024] over a 2-bank PSUM tile) to amortize the per-call overhead, and
the denominator is a DoubleRow ones-matmul on the PE (accumulated in a
[1,512] PSUM bank), keeping the DVE free for casts and the epilogue.
"""

import os
import sys

import numpy as np

for _p in ("/opt/trn_rl_repo", "/root/.axon_site/_ro/trn_rl_repo"):
    if os.path.isdir(_p) and _p not in sys.path:
        sys.path.insert(0, _p)

import concourse.bass as bass  # noqa: E402
import concourse.tile as tile  # noqa: E402
from concourse import bacc, mybir  # noqa: E402
from concourse.masks import make_identity  # noqa: E402

# The agent image's antenv lacks axon_hooks; if BASS_TRACE is set in the
# environment, run_bass_kernel_spmd would crash importing it. Provide a stub
# (profiling degrades gracefully to "hook isn't registered").
try:
    import antenv.axon_hooks  # noqa: F401
except ImportError:
    import types as _types

    _m = _types.ModuleType("antenv.axon_hooks")
    _h = [None]
    _m.set_axon_ntff_profile_hook = lambda h: _h.__setitem__(0, h)
    _m.get_axon_ntff_profile_hook = lambda: _h[0]
    sys.modules["antenv.axon_hooks"] = _m

B, C, H, W = 4, 256, 64, 64
N = H * W  # 4096 pixels
NQ = N // 2  # 2048 queries per core
G = 32  # groups
CPG = C // G  # 8 channels per group
EPS = 1e-5
NCORES = 8
SCALE = float(C) ** -0.5  # 0.0625
ESHIFT = -3.0  # exp(s*SCALE + ESHIFT): data max logit 7.95 -> P <= ~141 < 240
WS = 8.0  # weight fp8 pre-scale (entries ~N(0,1/16) need lifting)

F32 = mybir.dt.float32
BF16 = mybir.dt.bfloat16
FP8 = mybir.dt.float8e4

QB = 512  # query block (free dim of S^T / PV matmuls)
NQB = NQ // QB  # 4 query blocks
NKT = N // 128  # 32 key tiles
NKP = NKT // 2  # 16 key-tile pairs (DoubleRow granularity)
NNB = N // QB  # 8 pixel blocks for K/V projections
P = 128

DEBUG = bool(int(os.environ.get("KDEBUG", "0")))

Act = mybir.ActivationFunctionType
Alu = mybir.AluOpType
Axis = mybir.AxisListType
DR = mybir.MatmulPerfMode.DoubleRow

_NC = None
LAST_RESULTS = None


def _body(tc, d):
    nc = tc.nc
    x_d = d["x"]
    out_d = d["out"]

    const = tc.alloc_tile_pool(name="const", bufs=1)
    stage = tc.alloc_tile_pool(name="stage", bufs=2)
    small = tc.alloc_tile_pool(name="small", bufs=1)
    pblk = tc.alloc_tile_pool(name="pblk", bufs=2)
    work = tc.alloc_tile_pool(name="work", bufs=2)
    # PSUM budget (8 banks): S-pair tiles 2x[P,1024] = 4, aps 2x[P,512] = 2,
    # dps [1,512] = 1, po [P,512] = 1.
    ps = tc.alloc_tile_pool(name="ps", bufs=2, space="PSUM")
    ps_acc = tc.alloc_tile_pool(name="ps_acc", bufs=2, space="PSUM")
    ps_d = tc.alloc_tile_pool(name="ps_d", bufs=1, space="PSUM")
    ps_o = tc.alloc_tile_pool(name="ps_o", bufs=1, space="PSUM")

    # ---- x in SBUF first: bf16 [128, 2(ch), 4096] (residual + stats) and
    # fp8 [128, 2, 4096] (matmul operand), both cast on host; chunked so
    # bn_stats overlaps the transfer ----
    x_sb = const.tile([P, 2, N], BF16)
    x8 = const.tile([P, 2, N], FP8)
    x_src = x_d.ap().rearrange("(h p) n -> p h n", p=P)
    x8_src = d["x8"].ap().rearrange("(h p) n -> p h n", p=P)

    # PE warm-up: keep the HAM activity monitor busy during the DMA/stats
    # window so projections and attention run at full clock from the start.
    # Memsets go FIRST so the warm matmuls aren't queued behind the bn_stats
    # chain in DVE program order (PE would idle for the whole DMA phase).
    wu_w = const.tile([P, P], BF16)
    nc.vector.memset(wu_w, 0.0)
    wu_x = const.tile([P, 2 * P], BF16)
    nc.vector.memset(wu_x, 0.0)
    wu_ps = ps.tile([P, QB], F32, name="wu_ps", tag="mm")

    def warm(n):
        for _ in range(n):
            nc.tensor.matmul(
                wu_ps[:, 0:2 * P], lhsT=wu_w, rhs=wu_x, start=True, stop=True
            )

    warm(2)

    # group-reduce/broadcast masks (host-built): M1[p,g]=1 iff p//8==g,
    # M2 = M1^T. They turn the GroupNorm channel->group reduction and the
    # group->channel broadcast into two tiny PE matmuls (no transposes).
    m1_sb = const.tile([P, G // 2], F32)
    nc.sync.dma_start(out=m1_sb, in_=d["m1"][:, :])
    m2_sb = const.tile([G // 2, P], F32)
    nc.sync.dma_start(out=m2_sb, in_=d["m2"][:, :])
    # packed biases/affine: rows (bq,bk,bv,bo,gamma,beta) -> per-channel cols
    bcols = const.tile([P, 2, 6], F32)
    nc.gpsimd.dma_start(
        out=bcols, in_=d["bpack"].ap().rearrange("(h p) s -> p h s", p=P)
    )
    # all four weight matrices (bf16) in one DMA
    wstg = stage.tile([P, 4, 2, C], BF16, name="wstg", tag="wstg")
    nc.scalar.dma_start(
        out=wstg, in_=d["wpack"].ap().rearrange("(w h p) co -> p w h co", p=P, h=2)
    )

    # ---- head DMA: per-queue DMA bandwidth is only ~90GB/s, so the hot x8
    # (1MB, feeds stats AND all matmuls) is split over the three DMA-capable
    # queues (sync/ACT/gpsimd), while the bf16 x (2MB, residual-only, first
    # needed at the first epilogue) trickles in behind them.
    x8q = [nc.sync, nc.gpsimd, nc.scalar, nc.sync]
    for c in range(4):
        sl = (slice(None), slice(None), slice(c * 2 * QB, (c + 1) * 2 * QB))
        x8q[c].dma_start(out=x8[sl], in_=x8_src[sl])
    xsbq = [nc.sync, nc.gpsimd, nc.scalar, nc.sync]
    for c in range(4):
        sl = (slice(None), slice(None), slice(c * 2 * QB, (c + 1) * 2 * QB))
        xsbq[c].dma_start(out=x_sb[sl], in_=x_src[sl])

    # GroupNorm stats from the fp8 x (self-consistent: the projections
    # consume the same fp8 values; adds ~0.1% to the error budget). Split:
    # DVE bn_stats (ch0 + ch1 cols 0:1024), gpsimd sum + ACT sum-sq (rest
    # of ch1) so no single engine paces the head.
    bn_st0 = small.tile([P, NNB, 6], F32, name="bn_st0")
    bn_st1a = small.tile([P, 3, 6], F32, name="bn_st1a")
    sx1 = small.tile([P, 5], F32, name="sx1")
    sq1 = small.tile([P, 5], F32, name="sq1")
    scr = small.tile([P, QB], BF16, name="scr")
    for c in range(4):
        for k in range(2):
            j = 2 * c + k
            cols = slice(j * QB, (j + 1) * QB)
            nc.vector.bn_stats(out=bn_st0[:, j, :], in_=x8[:, 0, cols])
            if j < 3:
                nc.vector.bn_stats(out=bn_st1a[:, j, :], in_=x8[:, 1, cols])
            else:
                nc.scalar.activation(
                    scr, x8[:, 1, cols], Act.Identity,
                    accum_out=sx1[:, j - 3:j - 2],
                )
                nc.scalar.activation(
                    scr, x8[:, 1, cols], Act.Square,
                    accum_out=sq1[:, j - 3:j - 2],
                )
        warm(10)

    # ---- constants ----
    one11 = const.tile([1, 1], F32)
    nc.vector.memset(one11, 1.0)
    # padded to 16B so the DoubleRow k-tile stride meets walrus' step%16==0.
    # Value 1.0 balances the at_sb 1/8 down-scale and the x8 wo lift:
    # po*den_r = (8/8)*wo@at_un / den = wo@at_un/den.
    ones8_pad = const.tile([P, 2, 16], FP8)
    nc.vector.memset(ones8_pad, 1.0)
    ones8 = ones8_pad[:, :, 0:1]
    eps16 = const.tile([G // 2, 1], F32)
    nc.vector.memset(eps16, EPS)
    esh_col = const.tile([P, 1], F32)
    nc.vector.memset(esh_col, ESHIFT)
    # preload the ACT sqrt table; the exp set is loaded by a dummy exp right
    # after the dance so the switch overlaps the projection phase.
    warm11 = small.tile([1, 1], F32)
    nc.scalar.activation(warm11, one11, Act.Sqrt, scale=1.0)

    # ---- GroupNorm dance: per-channel (mean, E[x^2]) -> group reduce via
    # mask matmul -> rstd -> broadcast back via mask matmul -> a/b columns.
    mvex = small.tile([P, 2, 2], F32, name="mvex")  # (mean, ex2) per ch-half
    mv0 = small.tile([P, 2], F32, name="mv0")
    nc.vector.bn_aggr(out=mv0, in_=bn_st0)
    mv1a = small.tile([P, 2], F32, name="mv1a")
    nc.vector.bn_aggr(out=mv1a, in_=bn_st1a)
    tcol = small.tile([P, 6], F32, name="tcol")
    # ch0: mean, ex2 = var + mean^2
    nc.vector.tensor_copy(out=mvex[:, 0, 0:1], in_=mv0[:, 0:1])
    nc.vector.tensor_mul(tcol[:, 0:1], mv0[:, 0:1], mv0[:, 0:1])
    nc.vector.tensor_add(mvex[:, 0, 1:2], tcol[:, 0:1], mv0[:, 1:2])
    # ch1: combine bn_aggr over cols 0:1536 (weight 3/8) with the ACT
    # sums over cols 1024:4096
    nc.vector.tensor_reduce(out=tcol[:, 1:2], in_=sx1, axis=Axis.X, op=Alu.add)
    nc.vector.tensor_reduce(out=tcol[:, 2:3], in_=sq1, axis=Axis.X, op=Alu.add)
    nc.vector.tensor_scalar_mul(tcol[:, 3:4], tcol[:, 1:2], 1.0 / N)
    nc.vector.scalar_tensor_tensor(
        out=mvex[:, 1, 0:1], in0=mv1a[:, 0:1], scalar=0.375, in1=tcol[:, 3:4],
        op0=Alu.mult, op1=Alu.add,
    )
    nc.vector.tensor_mul(tcol[:, 4:5], mv1a[:, 0:1], mv1a[:, 0:1])
    nc.vector.tensor_add(tcol[:, 4:5], tcol[:, 4:5], mv1a[:, 1:2])
    nc.vector.tensor_scalar_mul(tcol[:, 5:6], tcol[:, 2:3], 1.0 / N)
    nc.vector.scalar_tensor_tensor(
        out=mvex[:, 1, 1:2], in0=tcol[:, 4:5], scalar=0.375, in1=tcol[:, 5:6],
        op0=Alu.mult, op1=Alu.add,
    )
    # group sums over partitions: [16 groups, (mean0, ex20, mean1, ex21)]
    gsum = ps_o.tile([G // 2, 4], F32, name="gsum", tag="po")
    nc.tensor.matmul(
        gsum, lhsT=m1_sb, rhs=mvex.rearrange("p c k -> p (c k)"),
        start=True, stop=True,
    )
    warm(4)
    vals = small.tile([G // 2, 4], F32, name="vals")  # (rstd0, rstd1, m0, m1)
    gtmp = small.tile([G // 2, 4], F32, name="gtmp")
    gview = gsum.rearrange("g (c k) -> g c k", k=2)
    nc.vector.tensor_scalar_mul(
        vals.rearrange("g (c k) -> g c k", k=2)[:, 1, :], gview[:, :, 0],
        1.0 / CPG,
    )
    nc.vector.tensor_scalar_mul(gtmp[:, 0:2], gview[:, :, 1], 1.0 / CPG)
    nc.vector.tensor_mul(gtmp[:, 2:4], vals[:, 2:4], vals[:, 2:4])
    nc.vector.tensor_sub(gtmp[:, 0:2], gtmp[:, 0:2], gtmp[:, 2:4])
    # rstd = 1/sqrt(var + eps) (sqrt table preloaded; recip on DVE)
    nc.scalar.activation(gtmp[:, 2:4], gtmp[:, 0:2], Act.Sqrt, bias=eps16)
    nc.vector.reciprocal(vals[:, 0:2], gtmp[:, 2:4])
    # broadcast back to channels: [128, (rstd0, rstd1, m0, m1)]
    bc = ps_acc.tile([P, 4], F32, name="bc", tag="acc")
    nc.tensor.matmul(bc, lhsT=m2_sb, rhs=vals, start=True, stop=True)
    warm(4)
    # a = gamma * rstd; b = beta - mean * a; a8 = 8a (per channel cols)
    cols8 = small.tile([P, 8], F32, name="cols8")
    a_cols = [cols8[:, 0:1], cols8[:, 1:2]]
    b_cols = [cols8[:, 2:3], cols8[:, 3:4]]
    a8_cols = [cols8[:, 4:5], cols8[:, 5:6]]
    for ch in range(2):
        nc.vector.tensor_mul(a_cols[ch], bcols[:, ch, 4:5], bc[:, ch:ch + 1])
        nc.vector.tensor_mul(cols8[:, 6 + ch:7 + ch], bc[:, 2 + ch:3 + ch],
                             a_cols[ch])
        nc.vector.tensor_sub(b_cols[ch], bcols[:, ch, 5:6],
                             cols8[:, 6 + ch:7 + ch])
        nc.vector.tensor_scalar_mul(a8_cols[ch], a_cols[ch], WS)

    # scale wq/wk/wv rows by 8*a (per input channel) straight from the f32
    # staging into fp8 tiles. The x8 lift keeps the fp8 entries
    # (~N(0, a/16)) out of e4m3 subnormal range; the PSUM->SBUF casts
    # divide it back out.
    w_s = {}
    for wi, wname in ((0, "wqt"), (1, "wkt"), (2, "wvt")):
        ws = const.tile([P, 2, C], FP8, name=f"{wname}_s")
        for ci in range(2):
            nc.vector.tensor_scalar_mul(
                ws[:, ci, :], wstg[:, wi, ci, :], a8_cols[ci]
            )
        w_s[wname] = ws
    # wo8 = 8*wo in fp8 (no GroupNorm folding on the out-proj)
    wo8 = const.tile([P, 2, C], FP8)
    for ch in range(2):
        nc.scalar.mul(wo8[:, ch, :], wstg[:, 3, ch, :], WS)

    # projection bias columns: be = W b + bias (bf16 matvecs off the staged
    # weights; psum ring across three pools so the chain pipelines)
    b_bf = small.tile([P, 2], BF16, name="b_bf")
    for ch in range(2):
        nc.vector.tensor_copy(out=b_bf[:, ch:ch + 1], in_=b_cols[ch])
    _mvi = [0]
    mv_pools = [(ps_o, "po"), (ps_acc, "acc"), (ps, "mm")]

    def matvec_bias(wi, rhs_cols, bias_idx, out_name, out_dt=F32):
        outs = []
        for co in range(2):
            pool, tag = mv_pools[_mvi[0] % 3]
            _mvi[0] += 1
            pe = pool.tile([P, 1], F32, name="pe_mv", tag=tag)
            for ci in range(2):
                nc.tensor.matmul(
                    pe, lhsT=wstg[:, wi, ci, co * P:(co + 1) * P],
                    rhs=rhs_cols[ci], start=(ci == 0), stop=(ci == 1),
                )
            t = small.tile([P, 1], out_dt, name=f"{out_name}_{co}")
            nc.scalar.activation(
                t, pe, Act.Identity, bias=bcols[:, co, bias_idx:bias_idx + 1],
                scale=1.0,
            )
            outs.append(t)
        return outs

    bcol_list = [b_bf[:, 0:1], b_bf[:, 1:2]]
    be_q = matvec_bias(0, bcol_list, 0, "be_q")
    be_k = matvec_bias(1, bcol_list, 1, "be_k")
    vbv = matvec_bias(2, bcol_list, 2, "vbv", out_dt=BF16)
    bo_eff = matvec_bias(3, vbv, 3, "bo_eff")
    warm(4)
    # dummy exp: pulls the exp table load into the projection phase
    nc.scalar.activation(warm11, one11, Act.Exp, scale=1.0)

    # ---- projections (all DoubleRow fp8, contraction over full C=256) ----
    # K^T [C, N] fp8: psum[co,nb] = sum_ci wkt8[ci,co].T @ x8[ci, nb] (x8)
    # and q,k = psum/8 + bias. Casts alternate ACT / DVE (ACT is idle until
    # the attention loop's exp stream starts).
    k_sb = const.tile([P, 2, N], FP8)
    q_sb = const.tile([P, 2, NQ], FP8)
    IWS = 1.0 / WS
    # casts rotate over three engines (ACT/DVE/Pool) and the psum tiles over
    # three pools (5 in-flight banks) so the MM stream never waits on a cast;
    # two-engine casting was the pacer that kept HAM throttled at K=4.
    proj_ps = [(ps, "mm"), (ps_acc, "acc"), (ps, "mm"), (ps_o, "po"),
               (ps_acc, "acc")]
    _pi = [0]

    def proj_tile():
        pool, tag = proj_ps[_pi[0] % len(proj_ps)]
        _pi[0] += 1
        return pool.tile([P, QB], F32, name="pp", tag=tag)

    _ci = [0]

    def proj_cast(out, in_, bias):
        # only ACT/DVE can read PSUM; the 5-bank psum ring above keeps the
        # MM stream ~2.5 tiles ahead so the 2-engine cast pace (~325ns/tile)
        # doesn't stall the PE.
        i = _ci[0] % 2
        _ci[0] += 1
        if i == 0:
            nc.scalar.activation(out, in_, Act.Identity, bias=bias, scale=IWS)
        else:
            nc.vector.tensor_scalar(
                out=out, in0=in_, scalar1=IWS, scalar2=bias, op0=Alu.mult,
                op1=Alu.add,
            )

    for nb in range(NNB):
        for co in range(2):
            if nb < NQB:
                pq = proj_tile()
                nc.tensor.matmul(
                    pq, lhsT=w_s["wqt"][:, :, co * P:(co + 1) * P],
                    rhs=x8[:, :, nb * QB:(nb + 1) * QB],
                    start=True, stop=True, perf_mode=DR,
                )
                proj_cast(q_sb[:, co, nb * QB:(nb + 1) * QB], pq, be_q[co])
            pk = proj_tile()
            nc.tensor.matmul(
                pk, lhsT=w_s["wkt"][:, :, co * P:(co + 1) * P],
                rhs=x8[:, :, nb * QB:(nb + 1) * QB],
                start=True, stop=True, perf_mode=DR,
            )
            proj_cast(k_sb[:, co, nb * QB:(nb + 1) * QB], pk, be_k[co])

    # V [N, C] fp8 (bias folded into bo_eff): psum[nt] = x8_tile.T @ wvt8
    v_sb = const.tile([P, NKT, C], FP8)
    v_flat = v_sb.rearrange("p k c -> p (k c)")
    zero_col = const.tile([P, 1], F32)
    nc.vector.memset(zero_col, 0.0)
    for nt in range(0, NKT, 2):
        pv = proj_tile()
        for n2 in range(2):
            nc.tensor.matmul(
                pv[:, n2 * C:(n2 + 1) * C],
                lhsT=x8[:, :, (nt + n2) * P:(nt + n2 + 1) * P],
                rhs=w_s["wvt"][:, :, :],
                start=True, stop=True, perf_mode=DR,
            )
        proj_cast(v_flat[:, nt * C:(nt + 2) * C], pv, zero_col)

    # ---- attention, per query block; DoubleRow over key-tile pairs with a
    # batched exp (one ACT call per pair reading a 2-bank PSUM tile) and the
    # denominator as a DoubleRow ones-matmul on the PE. The softmax division
    # is commuted through the out-projection: out = (wo8 @ PV) * (1/(8*den))
    # + bo_eff + x, deferred one qb so nothing waits on the reciprocal.
    def epilogue_a(qb, dps, aps, at_sb, p_sb_dbg=None):
        # casts first: they release the PV accumulator banks immediately.
        # 1/8 keeps the heavy-tailed PV numerator inside fp8's +-240.
        nc.vector.tensor_scalar_mul(at_sb[:, 0, :], aps[0], 1.0 / 8.0)
        nc.vector.tensor_scalar_mul(at_sb[:, 1, :], aps[1], 1.0 / 8.0)
        den_r = work.tile([1, QB], F32, name="den_r", tag="den_r")
        nc.vector.reciprocal_approx_fast(out=den_r, in_=dps)
        den_b = work.tile([P, QB], F32, name="den_b", tag="den_b", bufs=2)
        nc.gpsimd.partition_broadcast(den_b, den_r)
        if DEBUG:
            nc.sync.dma_start(
                out=d["dbg_denr"][:, qb * QB:(qb + 1) * QB], in_=den_r
            )
            nc.sync.dma_start(
                out=d["dbg_at"][:, qb * 2 * QB:(qb + 1) * 2 * QB],
                in_=at_sb.rearrange("p h n -> p (h n)"),
            )
            if qb == 0:
                nc.sync.dma_start(
                    out=d["dbg_p"][:, :], in_=p_sb_dbg.rearrange("p k n -> p (k n)")
                )
        return den_b

    def epilogue_co(qb, at_sb, den_b, co, po_pool=None, po_tag="po"):
        po_pool = po_pool or ps_o
        po = po_pool.tile([P, QB], F32, name="po", tag=po_tag)
        nc.tensor.matmul(
            po, lhsT=wo8[:, :, co * P:(co + 1) * P], rhs=at_sb,
            start=True, stop=True, perf_mode=DR,
        )
        t1 = work.tile([P, QB], F32, name="t1", tag="t1")
        nc.vector.tensor_mul(t1, po, den_b)
        res = work.tile([P, QB], F32, name="res", tag="res", bufs=4)
        nc.vector.scalar_tensor_tensor(
            out=res, in0=t1, scalar=bo_eff[co],
            in1=x_sb[:, co, qb * QB:(qb + 1) * QB], op0=Alu.add, op1=Alu.add,
        )
        nc.sync.dma_start(
            out=out_d[co * P:(co + 1) * P, qb * QB:(qb + 1) * QB], in_=res
        )

    if DEBUG:
        nc.sync.dma_start(out=d["dbg_k"][:, :], in_=k_sb.rearrange("p h n -> p (h n)"))
        nc.sync.dma_start(out=d["dbg_q"][:, :], in_=q_sb.rearrange("p h n -> p (h n)"))
        nc.sync.dma_start(out=d["dbg_v"][:, :], in_=v_flat)

    pending = None
    pend_den = None
    for qb in range(NQB):
        p_sb = pblk.tile([P, NKT, QB], FP8, name="p_sb")
        dps = ps_d.tile([1, QB], F32, name="dps")
        aps = [
            ps_acc.tile([P, QB], F32, name="aps", tag="acc") for _ in range(2)
        ]
        at_sb = work.tile([P, 2, QB], FP8, name="at_sb", tag="at_sb", bufs=2)
        for j in range(NKP + 2):
            if j == 1 and pending is not None:
                pend_den = epilogue_a(*pending)
            if j == 2 and pending is not None:
                epilogue_co(pending[0], pending[3], pend_den, 0)
            if j == 3 and pending is not None:
                epilogue_co(pending[0], pending[3], pend_den, 1)
                pending = None
            if j < NKP:
                sps2 = ps.tile([P, 2 * QB], F32, name="sps2", tag="mm")
                for h2 in range(2):
                    kt = 2 * j + h2
                    nc.tensor.matmul(
                        sps2[:, h2 * QB:(h2 + 1) * QB],
                        lhsT=k_sb[:, :, kt * P:(kt + 1) * P],
                        rhs=q_sb[:, :, qb * QB:(qb + 1) * QB],
                        start=True, stop=True, perf_mode=DR,
                    )
                nc.scalar.activation(
                    p_sb[:, 2 * j:2 * j + 2, :], sps2, Act.Exp,
                    bias=esh_col, scale=SCALE,
                )
            if j >= 2:
                pj = j - 2
                pair = p_sb[:, 2 * pj:2 * pj + 2, :]
                for ch in range(2):
                    nc.tensor.matmul(
                        aps[ch],
                        lhsT=v_sb[:, 2 * pj:2 * pj + 2, ch * P:(ch + 1) * P],
                        rhs=pair,
                        start=(pj == 0), stop=(pj == NKP - 1),
                        perf_mode=DR, skip_group_check=True,
                    )
                nc.tensor.matmul(
                    dps, lhsT=ones8, rhs=pair,
                    start=(pj == 0), stop=(pj == NKP - 1),
                    perf_mode=DR, skip_group_check=True,
                )
        pending = (qb, dps, aps, at_sb, p_sb)
    pend_den = epilogue_a(*pending)
    epilogue_co(pending[0], pending[3], pend_den, 0, po_pool=ps, po_tag="mm")
    epilogue_co(pending[0], pending[3], pend_den, 1, po_pool=ps, po_tag="mm")

    for pool in (ps_o, ps_d, ps_acc, ps, work, pblk, small, stage, const):
        pool.release()


def build_program():
    global _NC
    if _NC is not None:
        return _NC
    nc = bacc.Bacc("TRN2", target_bir_lowering=False, debug=False,
                   num_devices=NCORES)
    d = {
        "x": nc.dram_tensor("x", [C, N], BF16, kind="ExternalInput"),
        "x8": nc.dram_tensor("x8", [C, N], FP8, kind="ExternalInput"),
        "wpack": nc.dram_tensor("wpack", [4 * C, C], BF16, kind="ExternalInput"),
        "bpack": nc.dram_tensor("bpack", [C, 6], F32, kind="ExternalInput"),
        "m1": nc.dram_tensor("m1", [P, G // 2], F32, kind="ExternalInput"),
        "m2": nc.dram_tensor("m2", [G // 2, P], F32, kind="ExternalInput"),
        "out": nc.dram_tensor("out", [C, NQ], F32, kind="ExternalOutput"),
    }
    if DEBUG:
        d.update({
            "dbg_k": nc.dram_tensor("dbg_k", [P, 2 * N], FP8, kind="ExternalOutput"),
            "dbg_q": nc.dram_tensor("dbg_q", [P, 2 * NQ], FP8, kind="ExternalOutput"),
            "dbg_v": nc.dram_tensor("dbg_v", [P, NKT * C], FP8, kind="ExternalOutput"),
            "dbg_p": nc.dram_tensor("dbg_p", [P, NKT * QB], FP8, kind="ExternalOutput"),
            "dbg_denr": nc.dram_tensor("dbg_denr", [1, NQ], F32, kind="ExternalOutput"),
            "dbg_at": nc.dram_tensor("dbg_at", [P, NQB * 2 * QB], FP8, kind="ExternalOutput"),
        })
    with tile.TileContext(nc) as tc:
        _body(tc, d)
    nc.compile()
    _NC = nc
    return nc


def make_in_maps(x, gamma, beta, wq, bq, wk, bk, wv, bv, wo, bo):
    import ml_dtypes

    f32c = lambda a: np.ascontiguousarray(np.asarray(a, dtype=np.float32))
    x = f32c(x)
    wpack = np.concatenate(
        [f32c(np.asarray(w, np.float32).T) for w in (wq, wk, wv, wo)], axis=0
    )
    bpack = np.stack(
        [f32c(v).reshape(C) for v in (bq, bk, bv, bo, gamma, beta)], axis=1
    )
    m1 = np.zeros((P, G // 2), np.float32)
    for g in range(G // 2):
        m1[8 * g:8 * g + 8, g] = 1.0
    base = {
        "wpack": np.ascontiguousarray(wpack.astype(ml_dtypes.bfloat16)),
        "bpack": np.ascontiguousarray(bpack),
        "m1": m1,
        "m2": np.ascontiguousarray(m1.T),
    }
    import ml_dtypes

    in_maps = []
    for core in range(NCORES):
        b, h = divmod(core, 2)
        xb = x[b].reshape(C, N)
        if h:
            xb = np.concatenate([xb[:, NQ:], xb[:, :NQ]], axis=1)
        in_maps.append({
            **base,
            "x": np.ascontiguousarray(xb.astype(ml_dtypes.bfloat16)),
            "x8": np.ascontiguousarray(xb.astype(ml_dtypes.float8_e4m3)),
        })
    return in_maps


def kernel(x, gamma, beta, wq, bq, wk, bk, wv, bv, wo, bo):
    global LAST_RESULTS
    from concourse.bass_utils import run_bass_kernel_spmd

    nc = build_program()
    in_maps = make_in_maps(x, gamma, beta, wq, bq, wk, bk, wv, bv, wo, bo)
    res = run_bass_kernel_spmd(nc, in_maps, core_ids=list(range(NCORES)))
    LAST_RESULTS = res
    out = np.empty((B, C, N), np.float32)
    for core in range(NCORES):
        b, h = divmod(core, 2)
        out[b][:, h * NQ:(h + 1) * NQ] = res.results[core]["out"]
    return out.reshape(B, C, H, W)


# revision 34
# speedup vs baseline: 1.0454x; 1.0058x over previous
"""AttnBlock (GroupNorm + single-head self-attention + residual) on 8 TRN2 cores.

Sharding: data-parallel over (batch b, query-half h) -> 8 shards. Each core
receives the full [C, N] image of its batch (columns rolled so that its own
query half always occupies columns 0:NQ), computes GroupNorm stats + K/V over
the whole image, Q over its half, and a flash-style attention in which scores
are produced directly transposed (S^T = K^T.T @ Q^T tiles) so softmax
normalization is done via a ones-vector matmul and no PE transposes of P are
needed.

All large matmuls (projections, S^T, PV, denominator, out-proj) run as fp8e4
DoubleRow matmuls: lhsT [128, 2, M] / rhs [128, 2, N] contract 256 deep in a
single instruction at ~2x bf16 FLOP rate. Weights are scaled x8 before the
fp8 cast (their entries are ~N(0, 1/16) and would hit e4m3 subnormals); the
scale is removed in the PSUM->SBUF cast. The softmax exp is shifted by -2
(exp(s/16 - 2)) so P fits e4m3's +-240 range; numerator and denominator share
the shift so the ratio is unchanged. exp runs on ACT in 2-key-tile batches
([128,1024] over a 2-bank PSUM tile) to amortize the per-call overhead, and
the denominator is a DoubleRow ones-matmul on the PE (accumulated in a
[1,512] PSUM bank), keeping the DVE free for casts and the epilogue.
"""

import os
import sys

import numpy as np

for _p in ("/opt/trn_rl_repo", "/root/.axon_site/_ro/trn_rl_repo"):
    if os.path.isdir(_p) and _p not in sys.path:
        sys.path.insert(0, _p)

import concourse.bass as bass  # noqa: E402
import concourse.tile as tile  # noqa: E402
from concourse import bacc, mybir  # noqa: E402
from concourse.masks import make_identity  # noqa: E402

# The agent image's antenv lacks axon_hooks; if BASS_TRACE is set in the
# environment, run_bass_kernel_spmd would crash importing it. Provide a stub
# (profiling degrades gracefully to "hook isn't registered").
try:
    import antenv.axon_hooks  # noqa: F401
except ImportError:
    import types as _types

    _m = _types.ModuleType("antenv.axon_hooks")
    _h = [None]
    _m.set_axon_ntff_profile_hook = lambda h: _h.__setitem__(0, h)
    _m.get_axon_ntff_profile_hook = lambda: _h[0]
    sys.modules["antenv.axon_hooks"] = _m

B, C, H, W = 4, 256, 64, 64
N = H * W  # 4096 pixels
NQ = N // 2  # 2048 queries per core
G = 32  # groups
CPG = C // G  # 8 channels per group
EPS = 1e-5
NCORES = 8
SCALE = float(C) ** -0.5  # 0.0625
ESHIFT = -3.0  # exp(s*SCALE + ESHIFT): data max logit 7.95 -> P <= ~141 < 240
WS = 8.0  # weight fp8 pre-scale (entries ~N(0,1/16) need lifting)

F32 = mybir.dt.float32
BF16 = mybir.dt.bfloat16
FP8 = mybir.dt.float8e4

QB = 512  # query block (free dim of S^T / PV matmuls)
NQB = NQ // QB  # 4 query blocks
NKT = N // 128  # 32 key tiles
NKP = NKT // 2  # 16 key-tile pairs (DoubleRow granularity)
NNB = N // QB  # 8 pixel blocks for K/V projections
P = 128

DEBUG = bool(int(os.environ.get("KDEBUG", "0")))

Act = mybir.ActivationFunctionType
Alu = mybir.AluOpType
Axis = mybir.AxisListType
DR = mybir.MatmulPerfMode.DoubleRow

_NC = None
LAST_RESULTS = None


def _body(tc, d):
    nc = tc.nc
    x_d = d["x"]
    out_d = d["out"]

    const = tc.alloc_tile_pool(name="const", bufs=1)
    stage = tc.alloc_tile_pool(name="stage", bufs=2)
    small = tc.alloc_tile_pool(name="small", bufs=1)
    pblk = tc.alloc_tile_pool(name="pblk", bufs=2)
    work = tc.alloc_tile_pool(name="work", bufs=2)
    # PSUM budget (8 banks): S-pair tiles 2x[P,1024] = 4, aps 2x[P,512] = 2,
    # dps [1,512] = 1, po [P,512] = 1.
    ps = tc.alloc_tile_pool(name="ps", bufs=2, space="PSUM")
    ps_acc = tc.alloc_tile_pool(name="ps_acc", bufs=2, space="PSUM")
    ps_d = tc.alloc_tile_pool(name="ps_d", bufs=1, space="PSUM")
    ps_o = tc.alloc_tile_pool(name="ps_o", bufs=1, space="PSUM")

    # ---- x in SBUF first: bf16 [128, 2(ch), 4096] (residual + stats) and
    # fp8 [128, 2, 4096] (matmul operand), both cast on host; chunked so
    # bn_stats overlaps the transfer ----
    x_sb = const.tile([P, 2, N], BF16)
    x8 = const.tile([P, 2, N], FP8)
    x_src = x_d.ap().rearrange("(h p) n -> p h n", p=P)
    x8_src = d["x8"].ap().rearrange("(h p) n -> p h n", p=P)

    # PE warm-up: keep the HAM activity monitor busy during the DMA/stats
    # window so projections and attention run at full clock from the start.
    # Memsets go FIRST so the warm matmuls aren't queued behind the bn_stats
    # chain in DVE program order (PE would idle for the whole DMA phase).
    wu_w = const.tile([P, P], BF16)
    nc.vector.memset(wu_w, 0.0)
    wu_x = const.tile([P, 2 * P], BF16)
    nc.vector.memset(wu_x, 0.0)
    wu_ps = ps.tile([P, QB], F32, name="wu_ps", tag="mm")

    def warm(n):
        for _ in range(n):
            nc.tensor.matmul(
                wu_ps[:, 0:2 * P], lhsT=wu_w, rhs=wu_x, start=True, stop=True
            )

    warm(2)

    # group-reduce/broadcast masks (host-built): M1[p,g]=1 iff p//8==g,
    # M2 = M1^T. They turn the GroupNorm channel->group reduction and the
    # group->channel broadcast into two tiny PE matmuls (no transposes).
    m1_sb = const.tile([P, G // 2], F32)
    nc.sync.dma_start(out=m1_sb, in_=d["m1"][:, :])
    m2_sb = const.tile([G // 2, P], F32)
    nc.sync.dma_start(out=m2_sb, in_=d["m2"][:, :])
    # packed biases/affine: rows (bq,bk,bv,bo,gamma,beta) -> per-channel cols
    bcols = const.tile([P, 2, 6], F32)
    nc.gpsimd.dma_start(
        out=bcols, in_=d["bpack"].ap().rearrange("(h p) s -> p h s", p=P)
    )
    # all four weight matrices (bf16) in one DMA
    wstg = stage.tile([P, 4, 2, C], BF16, name="wstg", tag="wstg")
    nc.gpsimd.dma_start(
        out=wstg, in_=d["wpack"].ap().rearrange("(w h p) co -> p w h co", p=P, h=2)
    )

    # ---- head DMA: per-queue DMA bandwidth is only ~90GB/s, so the hot x8
    # (1MB, feeds stats AND all matmuls) is split over the three DMA-capable
    # queues (sync/ACT/gpsimd), while the bf16 x (2MB, residual-only, first
    # needed at the first epilogue) trickles in behind them.
    x8q = [nc.sync, nc.gpsimd, nc.scalar, nc.sync]  # noqa: line kept
    for c in range(4):
        sl = (slice(None), slice(None), slice(c * 2 * QB, (c + 1) * 2 * QB))
        x8q[c].dma_start(out=x8[sl], in_=x8_src[sl])
    xsbq = [nc.sync, nc.gpsimd, nc.scalar, nc.scalar]
    for c in range(4):
        sl = (slice(None), slice(None), slice(c * 2 * QB, (c + 1) * 2 * QB))
        xsbq[c].dma_start(out=x_sb[sl], in_=x_src[sl])

    # GroupNorm stats from the fp8 x (self-consistent: the projections
    # consume the same fp8 values; adds ~0.1% to the error budget). All on
    # DVE bn_stats: ACT must stay quiet so its sqrt table survives to the
    # dance (Identity/Square passes were thrashing the table sets).
    bn_st0 = small.tile([P, NNB, 6], F32, name="bn_st0")
    bn_st1 = small.tile([P, NNB, 6], F32, name="bn_st1")
    for c in range(4):
        for k in range(2):
            j = 2 * c + k
            cols = slice(j * QB, (j + 1) * QB)
            nc.vector.bn_stats(out=bn_st0[:, j, :], in_=x8[:, 0, cols])
            nc.vector.bn_stats(out=bn_st1[:, j, :], in_=x8[:, 1, cols])
        warm(10)

    # ---- constants ----
    one11 = const.tile([1, 1], F32)
    nc.vector.memset(one11, 1.0)
    # padded to 16B so the DoubleRow k-tile stride meets walrus' step%16==0.
    # Value 1.0 balances the at_sb 1/8 down-scale and the x8 wo lift:
    # po*den_r = (8/8)*wo@at_un / den = wo@at_un/den.
    ones8_pad = const.tile([P, 2, 16], FP8)
    nc.vector.memset(ones8_pad, 1.0)
    ones8 = ones8_pad[:, :, 0:1]
    eps16 = const.tile([G // 2, 1], F32)
    nc.vector.memset(eps16, EPS)
    esh_col = const.tile([P, 1], F32)
    nc.vector.memset(esh_col, ESHIFT)
    # preload the ACT sqrt table; the exp set is loaded by a dummy exp right
    # after the dance so the switch overlaps the projection phase.
    warm11 = small.tile([1, 1], F32)
    nc.scalar.activation(warm11, one11, Act.Sqrt, scale=1.0)

    # ---- GroupNorm dance: per-channel (mean, E[x^2]) -> group reduce via
    # mask matmul -> rstd -> broadcast back via mask matmul -> a/b columns.
    mvex = small.tile([P, 2, 2], F32, name="mvex")  # (mean, ex2) per ch-half
    mv0 = small.tile([P, 2], F32, name="mv0")
    nc.vector.bn_aggr(out=mv0, in_=bn_st0)
    mv1 = small.tile([P, 2], F32, name="mv1")
    nc.vector.bn_aggr(out=mv1, in_=bn_st1)
    tcol = small.tile([P, 6], F32, name="tcol")
    # ch0: mean, ex2 = var + mean^2
    nc.vector.tensor_copy(out=mvex[:, 0, 0:1], in_=mv0[:, 0:1])
    nc.vector.tensor_mul(tcol[:, 0:1], mv0[:, 0:1], mv0[:, 0:1])
    nc.vector.tensor_add(mvex[:, 0, 1:2], tcol[:, 0:1], mv0[:, 1:2])
    # ch1: mean, ex2 = var + mean^2
    nc.vector.tensor_copy(out=mvex[:, 1, 0:1], in_=mv1[:, 0:1])
    nc.vector.tensor_mul(tcol[:, 1:2], mv1[:, 0:1], mv1[:, 0:1])
    nc.vector.tensor_add(mvex[:, 1, 1:2], tcol[:, 1:2], mv1[:, 1:2])
    # group sums over partitions: [16 groups, (mean0, ex20, mean1, ex21)]
    gsum = ps_o.tile([G // 2, 4], F32, name="gsum", tag="po")
    nc.tensor.matmul(
        gsum, lhsT=m1_sb, rhs=mvex.rearrange("p c k -> p (c k)"),
        start=True, stop=True,
    )
    warm(4)
    vals = small.tile([G // 2, 4], F32, name="vals")  # (rstd0, rstd1, m0, m1)
    gtmp = small.tile([G // 2, 4], F32, name="gtmp")
    gview = gsum.rearrange("g (c k) -> g c k", k=2)
    nc.vector.tensor_scalar_mul(
        vals.rearrange("g (c k) -> g c k", k=2)[:, 1, :], gview[:, :, 0],
        1.0 / CPG,
    )
    nc.vector.tensor_scalar_mul(gtmp[:, 0:2], gview[:, :, 1], 1.0 / CPG)
    nc.vector.tensor_mul(gtmp[:, 2:4], vals[:, 2:4], vals[:, 2:4])
    nc.vector.tensor_sub(gtmp[:, 0:2], gtmp[:, 0:2], gtmp[:, 2:4])
    # rstd = 1/sqrt(var + eps) (sqrt table preloaded; recip on DVE)
    nc.scalar.activation(gtmp[:, 2:4], gtmp[:, 0:2], Act.Sqrt, bias=eps16)
    nc.vector.reciprocal(vals[:, 0:2], gtmp[:, 2:4])
    # broadcast back to channels: [128, (rstd0, rstd1, m0, m1)]
    bc = ps_acc.tile([P, 4], F32, name="bc", tag="acc")
    nc.tensor.matmul(bc, lhsT=m2_sb, rhs=vals, start=True, stop=True)
    warm(4)
    # a = gamma * rstd; b = beta - mean * a; a8 = 8a (per channel cols)
    cols8 = small.tile([P, 8], F32, name="cols8")
    a_cols = [cols8[:, 0:1], cols8[:, 1:2]]
    b_cols = [cols8[:, 2:3], cols8[:, 3:4]]
    a8_cols = [cols8[:, 4:5], cols8[:, 5:6]]
    for ch in range(2):
        nc.vector.tensor_mul(a_cols[ch], bcols[:, ch, 4:5], bc[:, ch:ch + 1])
        nc.vector.tensor_mul(cols8[:, 6 + ch:7 + ch], bc[:, 2 + ch:3 + ch],
                             a_cols[ch])
        nc.vector.tensor_sub(b_cols[ch], bcols[:, ch, 5:6],
                             cols8[:, 6 + ch:7 + ch])
        nc.vector.tensor_scalar_mul(a8_cols[ch], a_cols[ch], WS)

    # scale wq/wk/wv rows by 8*a (per input channel) straight from the f32
    # staging into fp8 tiles. The x8 lift keeps the fp8 entries
    # (~N(0, a/16)) out of e4m3 subnormal range; the PSUM->SBUF casts
    # divide it back out.
    w_s = {}
    for wi, wname in ((0, "wqt"), (1, "wkt"), (2, "wvt")):
        ws = const.tile([P, 2, C], FP8, name=f"{wname}_s")
        for ci in range(2):
            nc.vector.tensor_scalar_mul(
                ws[:, ci, :], wstg[:, wi, ci, :], a8_cols[ci]
            )
        w_s[wname] = ws
    # wo8 = 8*wo in fp8 (no GroupNorm folding on the out-proj)
    wo8 = const.tile([P, 2, C], FP8)
    for ch in range(2):
        nc.scalar.mul(wo8[:, ch, :], wstg[:, 3, ch, :], WS)

    # projection bias columns: be = W b + bias (bf16 matvecs off the staged
    # weights; psum ring across three pools so the chain pipelines)
    b_bf = small.tile([P, 2], BF16, name="b_bf")
    for ch in range(2):
        nc.vector.tensor_copy(out=b_bf[:, ch:ch + 1], in_=b_cols[ch])
    _mvi = [0]
    mv_pools = [(ps_o, "po"), (ps_acc, "acc"), (ps, "mm")]

    def matvec_bias(wi, rhs_cols, bias_idx, out_name, out_dt=F32):
        outs = []
        for co in range(2):
            pool, tag = mv_pools[_mvi[0] % 3]
            _mvi[0] += 1
            pe = pool.tile([P, 1], F32, name="pe_mv", tag=tag)
            for ci in range(2):
                nc.tensor.matmul(
                    pe, lhsT=wstg[:, wi, ci, co * P:(co + 1) * P],
                    rhs=rhs_cols[ci], start=(ci == 0), stop=(ci == 1),
                )
            t = small.tile([P, 1], out_dt, name=f"{out_name}_{co}")
            nc.scalar.activation(
                t, pe, Act.Identity, bias=bcols[:, co, bias_idx:bias_idx + 1],
                scale=1.0,
            )
            outs.append(t)
        return outs

    bcol_list = [b_bf[:, 0:1], b_bf[:, 1:2]]
    be_q = matvec_bias(0, bcol_list, 0, "be_q")
    be_k = matvec_bias(1, bcol_list, 1, "be_k")
    vbv = matvec_bias(2, bcol_list, 2, "vbv", out_dt=BF16)
    bo_eff = matvec_bias(3, vbv, 3, "bo_eff")
    warm(4)
    # dummy exp: pulls the exp table load into the projection phase
    nc.scalar.activation(warm11, one11, Act.Exp, scale=1.0)

    # ---- projections (all DoubleRow fp8, contraction over full C=256) ----
    # K^T [C, N] fp8: psum[co,nb] = sum_ci wkt8[ci,co].T @ x8[ci, nb] (x8)
    # and q,k = psum/8 + bias. Casts alternate ACT / DVE (ACT is idle until
    # the attention loop's exp stream starts).
    k_sb = const.tile([P, 2, N], FP8)
    q_sb = const.tile([P, 2, NQ], FP8)
    IWS = 1.0 / WS
    # casts rotate over three engines (ACT/DVE/Pool) and the psum tiles over
    # three pools (5 in-flight banks) so the MM stream never waits on a cast;
    # two-engine casting was the pacer that kept HAM throttled at K=4.
    proj_ps = [(ps, "mm"), (ps_acc, "acc"), (ps, "mm"), (ps_o, "po"),
               (ps_acc, "acc")]
    _pi = [0]

    def proj_tile():
        pool, tag = proj_ps[_pi[0] % len(proj_ps)]
        _pi[0] += 1
        return pool.tile([P, QB], F32, name="pp", tag=tag)

    _ci = [0]

    def proj_cast(out, in_, bias):
        # only ACT/DVE can read PSUM; the 5-bank psum ring above keeps the
        # MM stream ~2.5 tiles ahead so the 2-engine cast pace (~325ns/tile)
        # doesn't stall the PE.
        i = _ci[0] % 2
        _ci[0] += 1
        if i == 0:
            nc.scalar.activation(out, in_, Act.Identity, bias=bias, scale=IWS)
        else:
            nc.vector.tensor_scalar(
                out=out, in0=in_, scalar1=IWS, scalar2=bias, op0=Alu.mult,
                op1=Alu.add,
            )

    for nb in range(NNB):
        for co in range(2):
            if nb < NQB:
                pq = proj_tile()
                nc.tensor.matmul(
                    pq, lhsT=w_s["wqt"][:, :, co * P:(co + 1) * P],
                    rhs=x8[:, :, nb * QB:(nb + 1) * QB],
                    start=True, stop=True, perf_mode=DR,
                )
                proj_cast(q_sb[:, co, nb * QB:(nb + 1) * QB], pq, be_q[co])
            pk = proj_tile()
            nc.tensor.matmul(
                pk, lhsT=w_s["wkt"][:, :, co * P:(co + 1) * P],
                rhs=x8[:, :, nb * QB:(nb + 1) * QB],
                start=True, stop=True, perf_mode=DR,
            )
            proj_cast(k_sb[:, co, nb * QB:(nb + 1) * QB], pk, be_k[co])

    # V [N, C] fp8 (bias folded into bo_eff): psum[nt] = x8_tile.T @ wvt8
    v_sb = const.tile([P, NKT, C], FP8)
    v_flat = v_sb.rearrange("p k c -> p (k c)")
    zero_col = const.tile([P, 1], F32)
    nc.vector.memset(zero_col, 0.0)
    for nt in range(0, NKT, 2):
        pv = proj_tile()
        for n2 in range(2):
            nc.tensor.matmul(
                pv[:, n2 * C:(n2 + 1) * C],
                lhsT=x8[:, :, (nt + n2) * P:(nt + n2 + 1) * P],
                rhs=w_s["wvt"][:, :, :],
                start=True, stop=True, perf_mode=DR,
            )
        proj_cast(v_flat[:, nt * C:(nt + 2) * C], pv, zero_col)

    # ---- attention, per query block; DoubleRow over key-tile pairs with a
    # batched exp (one ACT call per pair reading a 2-bank PSUM tile) and the
    # denominator as a DoubleRow ones-matmul on the PE. The softmax division
    # is commuted through the out-projection: out = (wo8 @ PV) * (1/(8*den))
    # + bo_eff + x, deferred one qb so nothing waits on the reciprocal.
    def epilogue_a(qb, dps, aps, at_sb, p_sb_dbg=None):
        # casts first: they release the PV accumulator banks immediately.
        # 1/8 keeps the heavy-tailed PV numerator inside fp8's +-240.
        nc.vector.tensor_scalar_mul(at_sb[:, 0, :], aps[0], 1.0 / 8.0)
        nc.vector.tensor_scalar_mul(at_sb[:, 1, :], aps[1], 1.0 / 8.0)
        den_r = work.tile([1, QB], F32, name="den_r", tag="den_r")
        nc.vector.reciprocal_approx_fast(out=den_r, in_=dps)
        den_b = work.tile([P, QB], F32, name="den_b", tag="den_b", bufs=2)
        nc.gpsimd.partition_broadcast(den_b, den_r)
        if DEBUG:
            nc.sync.dma_start(
                out=d["dbg_denr"][:, qb * QB:(qb + 1) * QB], in_=den_r
            )
            nc.sync.dma_start(
                out=d["dbg_at"][:, qb * 2 * QB:(qb + 1) * 2 * QB],
                in_=at_sb.rearrange("p h n -> p (h n)"),
            )
            if qb == 0:
                nc.sync.dma_start(
                    out=d["dbg_p"][:, :], in_=p_sb_dbg.rearrange("p k n -> p (k n)")
                )
        return den_b

    def epilogue_co(qb, at_sb, den_b, co, po_pool=None, po_tag="po"):
        po_pool = po_pool or ps_o
        po = po_pool.tile([P, QB], F32, name="po", tag=po_tag)
        nc.tensor.matmul(
            po, lhsT=wo8[:, :, co * P:(co + 1) * P], rhs=at_sb,
            start=True, stop=True, perf_mode=DR,
        )
        t1 = work.tile([P, QB], F32, name="t1", tag="t1")
        nc.vector.tensor_mul(t1, po, den_b)
        res = work.tile([P, QB], F32, name="res", tag="res", bufs=4)
        nc.vector.scalar_tensor_tensor(
            out=res, in0=t1, scalar=bo_eff[co],
            in1=x_sb[:, co, qb * QB:(qb + 1) * QB], op0=Alu.add, op1=Alu.add,
        )
        nc.sync.dma_start(
            out=out_d[co * P:(co + 1) * P, qb * QB:(qb + 1) * QB], in_=res
        )

    if DEBUG:
        nc.sync.dma_start(out=d["dbg_k"][:, :], in_=k_sb.rearrange("p h n -> p (h n)"))
        nc.sync.dma_start(out=d["dbg_q"][:, :], in_=q_sb.rearrange("p h n -> p (h n)"))
        nc.sync.dma_start(out=d["dbg_v"][:, :], in_=v_flat)

    pending = None
    pend_den = None
    for qb in range(NQB):
        p_sb = pblk.tile([P, NKT, QB], FP8, name="p_sb")
        dps = ps_d.tile([1, QB], F32, name="dps")
        aps = [
            ps_acc.tile([P, QB], F32, name="aps", tag="acc") for _ in range(2)
        ]
        at_sb = work.tile([P, 2, QB], FP8, name="at_sb", tag="at_sb", bufs=2)
        for j in range(NKP + 2):
            if j == 1 and pending is not None:
                pend_den = epilogue_a(*pending)
            if j == 2 and pending is not None:
                epilogue_co(pending[0], pending[3], pend_den, 0)
            if j == 3 and pending is not None:
                epilogue_co(pending[0], pending[3], pend_den, 1)
                pending = None
            if j < NKP:
                sps2 = ps.tile([P, 2 * QB], F32, name="sps2", tag="mm")
                for h2 in range(2):
                    kt = 2 * j + h2
                    nc.tensor.matmul(
                        sps2[:, h2 * QB:(h2 + 1) * QB],
                        lhsT=k_sb[:, :, kt * P:(kt + 1) * P],
                        rhs=q_sb[:, :, qb * QB:(qb + 1) * QB],
                        start=True, stop=True, perf_mode=DR,
                    )
                nc.scalar.activation(
                    p_sb[:, 2 * j:2 * j + 2, :], sps2, Act.Exp,
                    bias=esh_col, scale=SCALE,
                )
            if j >= 2:
                pj = j - 2
                pair = p_sb[:, 2 * pj:2 * pj + 2, :]
                for ch in range(2):
                    nc.tensor.matmul(
                        aps[ch],
                        lhsT=v_sb[:, 2 * pj:2 * pj + 2, ch * P:(ch + 1) * P],
                        rhs=pair,
                        start=(pj == 0), stop=(pj == NKP - 1),
                        perf_mode=DR, skip_group_check=True,
                    )
                nc.tensor.matmul(
                    dps, lhsT=ones8, rhs=pair,
                    start=(pj == 0), stop=(pj == NKP - 1),
                    perf_mode=DR, skip_group_check=True,
                )
        pending = (qb, dps, aps, at_sb, p_sb)
    pend_den = epilogue_a(*pending)
    epilogue_co(pending[0], pending[3], pend_den, 0, po_pool=ps, po_tag="mm")
    epilogue_co(pending[0], pending[3], pend_den, 1, po_pool=ps, po_tag="mm")

    for pool in (ps_o, ps_d, ps_acc, ps, work, pblk, small, stage, const):
        pool.release()


def build_program():
    global _NC
    if _NC is not None:
        return _NC
    nc = bacc.Bacc("TRN2", target_bir_lowering=False, debug=False,
                   num_devices=NCORES)
    d = {
        "x": nc.dram_tensor("x", [C, N], BF16, kind="ExternalInput"),
        "x8": nc.dram_tensor("x8", [C, N], FP8, kind="ExternalInput"),
        "wpack": nc.dram_tensor("wpack", [4 * C, C], BF16, kind="ExternalInput"),
        "bpack": nc.dram_tensor("bpack", [C, 6], F32, kind="ExternalInput"),
        "m1": nc.dram_tensor("m1", [P, G // 2], F32, kind="ExternalInput"),
        "m2": nc.dram_tensor("m2", [G // 2, P], F32, kind="ExternalInput"),
        "out": nc.dram_tensor("out", [C, NQ], F32, kind="ExternalOutput"),
    }
    if DEBUG:
        d.update({
            "dbg_k": nc.dram_tensor("dbg_k", [P, 2 * N], FP8, kind="ExternalOutput"),
            "dbg_q": nc.dram_tensor("dbg_q", [P, 2 * NQ], FP8, kind="ExternalOutput"),
            "dbg_v": nc.dram_tensor("dbg_v", [P, NKT * C], FP8, kind="ExternalOutput"),
            "dbg_p": nc.dram_tensor("dbg_p", [P, NKT * QB], FP8, kind="ExternalOutput"),
            "dbg_denr": nc.dram_tensor("dbg_denr", [1, NQ], F32, kind="ExternalOutput"),
            "dbg_at": nc.dram_tensor("dbg_at", [P, NQB * 2 * QB], FP8, kind="ExternalOutput"),
        })
    with tile.TileContext(nc) as tc:
        _body(tc, d)
    nc.compile()
    _NC = nc
    return nc


def make_in_maps(x, gamma, beta, wq, bq, wk, bk, wv, bv, wo, bo):
    import ml_dtypes

    f32c = lambda a: np.ascontiguousarray(np.asarray(a, dtype=np.float32))
    x = f32c(x)
    wpack = np.concatenate(
        [f32c(np.asarray(w, np.float32).T) for w in (wq, wk, wv, wo)], axis=0
    )
    bpack = np.stack(
        [f32c(v).reshape(C) for v in (bq, bk, bv, bo, gamma, beta)], axis=1
    )
    m1 = np.zeros((P, G // 2), np.float32)
    for g in range(G // 2):
        m1[8 * g:8 * g + 8, g] = 1.0
    base = {
        "wpack": np.ascontiguousarray(wpack.astype(ml_dtypes.bfloat16)),
        "bpack": np.ascontiguousarray(bpack),
        "m1": m1,
        "m2": np.ascontiguousarray(m1.T),
    }
    import ml_dtypes

    in_maps = []
    for core in range(NCORES):
        b, h = divmod(core, 2)
        xb = x[b].reshape(C, N)
        if h:
            xb = np.concatenate([xb[:, NQ:], xb[:, :NQ]], axis=1)
        in_maps.append({
            **base,
            "x": np.ascontiguousarray(xb.astype(ml_dtypes.bfloat16)),
            "x8": np.ascontiguousarray(xb.astype(ml_dtypes.float8_e4m3)),
        })
    return in_maps


def kernel(x, gamma, beta, wq, bq, wk, bk, wv, bv, wo, bo):
    global LAST_RESULTS
    from concourse.bass_utils import run_bass_kernel_spmd

    nc = build_program()
    in_maps = make_in_maps(x, gamma, beta, wq, bq, wk, bk, wv, bv, wo, bo)
    res = run_bass_kernel_spmd(nc, in_maps, core_ids=list(range(NCORES)))
    LAST_RESULTS = res
    out = np.empty((B, C, N), np.float32)
    for core in range(NCORES):
        b, h = divmod(core, 2)
        out[b][:, h * NQ:(h + 1) * NQ] = res.results[core]["out"]
    return out.reshape(B, C, H, W)


# revision 35
# speedup vs baseline: 1.0784x; 1.0315x over previous
"""AttnBlock (GroupNorm + single-head self-attention + residual) on 8 TRN2 cores.

Sharding: data-parallel over (batch b, query-half h) -> 8 shards. Each core
receives the full [C, N] image of its batch (columns rolled so that its own
query half always occupies columns 0:NQ), computes GroupNorm stats + K/V over
the whole image, Q over its half, and a flash-style attention in which scores
are produced directly transposed (S^T = K^T.T @ Q^T tiles) so softmax
normalization is done via a ones-vector matmul and no PE transposes of P are
needed.

All large matmuls (projections, S^T, PV, denominator, out-proj) run as fp8e4
DoubleRow matmuls: lhsT [128, 2, M] / rhs [128, 2, N] contract 256 deep in a
single instruction at ~2x bf16 FLOP rate. Weights are scaled x8 before the
fp8 cast (their entries are ~N(0, 1/16) and would hit e4m3 subnormals); the
scale is removed in the PSUM->SBUF cast. The softmax exp is shifted by -2
(exp(s/16 - 2)) so P fits e4m3's +-240 range; numerator and denominator share
the shift so the ratio is unchanged. exp runs on ACT in 2-key-tile batches
([128,1024] over a 2-bank PSUM tile) to amortize the per-call overhead, and
the denominator is a DoubleRow ones-matmul on the PE (accumulated in a
[1,512] PSUM bank), keeping the DVE free for casts and the epilogue.
"""

import os
import sys

import numpy as np

for _p in ("/opt/trn_rl_repo", "/root/.axon_site/_ro/trn_rl_repo"):
    if os.path.isdir(_p) and _p not in sys.path:
        sys.path.insert(0, _p)

import concourse.bass as bass  # noqa: E402
import concourse.tile as tile  # noqa: E402
from concourse import bacc, mybir  # noqa: E402
from concourse.masks import make_identity  # noqa: E402

# The agent image's antenv lacks axon_hooks; if BASS_TRACE is set in the
# environment, run_bass_kernel_spmd would crash importing it. Provide a stub
# (profiling degrades gracefully to "hook isn't registered").
try:
    import antenv.axon_hooks  # noqa: F401
except ImportError:
    import types as _types

    _m = _types.ModuleType("antenv.axon_hooks")
    _h = [None]
    _m.set_axon_ntff_profile_hook = lambda h: _h.__setitem__(0, h)
    _m.get_axon_ntff_profile_hook = lambda: _h[0]
    sys.modules["antenv.axon_hooks"] = _m

B, C, H, W = 4, 256, 64, 64
N = H * W  # 4096 pixels
NQ = N // 2  # 2048 queries per core
G = 32  # groups
CPG = C // G  # 8 channels per group
EPS = 1e-5
NCORES = 8
SCALE = float(C) ** -0.5  # 0.0625
ESHIFT = -3.0  # exp(s*SCALE + ESHIFT): data max logit 7.95 -> P <= ~141 < 240
WS = 8.0  # weight fp8 pre-scale (entries ~N(0,1/16) need lifting)

F32 = mybir.dt.float32
BF16 = mybir.dt.bfloat16
FP8 = mybir.dt.float8e4

QB = 512  # query block (free dim of S^T / PV matmuls)
NQB = NQ // QB  # 4 query blocks
NKT = N // 128  # 32 key tiles
NKP = NKT // 2  # 16 key-tile pairs (DoubleRow granularity)
NNB = N // QB  # 8 pixel blocks for K/V projections
P = 128

DEBUG = bool(int(os.environ.get("KDEBUG", "0")))

Act = mybir.ActivationFunctionType
Alu = mybir.AluOpType
Axis = mybir.AxisListType
DR = mybir.MatmulPerfMode.DoubleRow

_NC = None
LAST_RESULTS = None


def _body(tc, d):
    nc = tc.nc
    x_d = d["x"]
    out_d = d["out"]

    const = tc.alloc_tile_pool(name="const", bufs=1)
    stage = tc.alloc_tile_pool(name="stage", bufs=2)
    small = tc.alloc_tile_pool(name="small", bufs=1)
    pblk = tc.alloc_tile_pool(name="pblk", bufs=2)
    work = tc.alloc_tile_pool(name="work", bufs=2)
    # PSUM budget (8 banks): S-pair tiles 2x[P,1024] = 4, aps 2x[P,512] = 2,
    # dps [1,512] = 1, po [P,512] = 1.
    ps = tc.alloc_tile_pool(name="ps", bufs=2, space="PSUM")
    ps_acc = tc.alloc_tile_pool(name="ps_acc", bufs=2, space="PSUM")
    ps_d = tc.alloc_tile_pool(name="ps_d", bufs=1, space="PSUM")
    ps_o = tc.alloc_tile_pool(name="ps_o", bufs=1, space="PSUM")

    # ---- x in SBUF first: bf16 [128, 2(ch), 4096] (residual + stats) and
    # fp8 [128, 2, 4096] (matmul operand), both cast on host; chunked so
    # bn_stats overlaps the transfer ----
    x_sb = const.tile([P, 2, N], BF16)
    x8 = const.tile([P, 2, N], FP8)
    x_src = x_d.ap().rearrange("(h p) n -> p h n", p=P)
    x8_src = d["x8"].ap().rearrange("(h p) n -> p h n", p=P)

    # PE warm-up: keep the HAM activity monitor busy during the DMA/stats
    # window so projections and attention run at full clock from the start.
    # Memsets go FIRST so the warm matmuls aren't queued behind the bn_stats
    # chain in DVE program order (PE would idle for the whole DMA phase).
    wu_w = const.tile([P, P], BF16)
    nc.vector.memset(wu_w, 0.0)
    wu_x = const.tile([P, 2 * P], BF16)
    nc.vector.memset(wu_x, 0.0)
    wu_ps = ps.tile([P, QB], F32, name="wu_ps", tag="mm")

    def warm(n):
        for _ in range(n):
            nc.tensor.matmul(
                wu_ps[:, 0:2 * P], lhsT=wu_w, rhs=wu_x, start=True, stop=True
            )

    warm(2)

    # group-reduce/broadcast masks (host-built): M1[p,g]=1 iff p//8==g,
    # M2 = M1^T. They turn the GroupNorm channel->group reduction and the
    # group->channel broadcast into two tiny PE matmuls (no transposes).
    m1_sb = const.tile([P, G // 2], F32)
    nc.sync.dma_start(out=m1_sb, in_=d["m1"][:, :])
    m2_sb = const.tile([G // 2, P], F32)
    nc.sync.dma_start(out=m2_sb, in_=d["m2"][:, :])
    # packed biases/affine: rows (bq,bk,bv,bo,gamma,beta) -> per-channel cols
    bcols = const.tile([P, 2, 6], F32)
    nc.gpsimd.dma_start(
        out=bcols, in_=d["bpack"].ap().rearrange("(h p) s -> p h s", p=P)
    )
    # ---- head DMA: per-queue DMA bandwidth is only ~90GB/s, so the hot x8
    # (1MB, feeds stats AND all matmuls) is split over the three DMA-capable
    # queues (sync/ACT/gpsimd), while the weight pack (needed ~15us later)
    # and the bf16 x (2MB, residual-only, first needed at the first
    # epilogue) queue up behind them.
    x8q = [nc.sync, nc.gpsimd, nc.scalar, nc.sync]
    for c in range(4):
        sl = (slice(None), slice(None), slice(c * 2 * QB, (c + 1) * 2 * QB))
        x8q[c].dma_start(out=x8[sl], in_=x8_src[sl])
    # all four weight matrices (bf16) in one DMA, behind x8 c1 on gpsimd
    wstg = stage.tile([P, 4, 2, C], BF16, name="wstg", tag="wstg")
    nc.gpsimd.dma_start(
        out=wstg, in_=d["wpack"].ap().rearrange("(w h p) co -> p w h co", p=P, h=2)
    )
    xsbq = [nc.sync, nc.gpsimd, nc.scalar, nc.scalar]
    for c in range(4):
        sl = (slice(None), slice(None), slice(c * 2 * QB, (c + 1) * 2 * QB))
        xsbq[c].dma_start(out=x_sb[sl], in_=x_src[sl])

    # GroupNorm stats from the fp8 x (self-consistent: the projections
    # consume the same fp8 values; adds ~0.1% to the error budget). All on
    # DVE bn_stats: ACT must stay quiet so its sqrt table survives to the
    # dance (Identity/Square passes were thrashing the table sets).
    bn_st0 = small.tile([P, NNB, 6], F32, name="bn_st0")
    bn_st1 = small.tile([P, NNB, 6], F32, name="bn_st1")
    for c in range(4):
        for k in range(2):
            j = 2 * c + k
            cols = slice(j * QB, (j + 1) * QB)
            nc.vector.bn_stats(out=bn_st0[:, j, :], in_=x8[:, 0, cols])
            nc.vector.bn_stats(out=bn_st1[:, j, :], in_=x8[:, 1, cols])
        warm(10)

    # ---- constants ----
    one11 = const.tile([1, 1], F32)
    nc.vector.memset(one11, 1.0)
    # padded to 16B so the DoubleRow k-tile stride meets walrus' step%16==0.
    # Value 1.0 balances the at_sb 1/8 down-scale and the x8 wo lift:
    # po*den_r = (8/8)*wo@at_un / den = wo@at_un/den.
    ones8_pad = const.tile([P, 2, 16], FP8)
    nc.vector.memset(ones8_pad, 1.0)
    ones8 = ones8_pad[:, :, 0:1]
    eps16 = const.tile([G // 2, 1], F32)
    nc.vector.memset(eps16, EPS)
    esh_col = const.tile([P, 1], F32)
    nc.vector.memset(esh_col, ESHIFT)
    # preload the ACT sqrt table; the exp set is loaded by a dummy exp right
    # after the dance so the switch overlaps the projection phase.
    warm11 = small.tile([1, 1], F32)
    nc.scalar.activation(warm11, one11, Act.Sqrt, scale=1.0)

    # ---- GroupNorm dance: per-channel (mean, E[x^2]) -> group reduce via
    # mask matmul -> rstd -> broadcast back via mask matmul -> a/b columns.
    mvex = small.tile([P, 2, 2], F32, name="mvex")  # (mean, ex2) per ch-half
    mv0 = small.tile([P, 2], F32, name="mv0")
    nc.vector.bn_aggr(out=mv0, in_=bn_st0)
    mv1 = small.tile([P, 2], F32, name="mv1")
    nc.vector.bn_aggr(out=mv1, in_=bn_st1)
    tcol = small.tile([P, 6], F32, name="tcol")
    # ch0: mean, ex2 = var + mean^2
    nc.vector.tensor_copy(out=mvex[:, 0, 0:1], in_=mv0[:, 0:1])
    nc.vector.tensor_mul(tcol[:, 0:1], mv0[:, 0:1], mv0[:, 0:1])
    nc.vector.tensor_add(mvex[:, 0, 1:2], tcol[:, 0:1], mv0[:, 1:2])
    # ch1: mean, ex2 = var + mean^2
    nc.vector.tensor_copy(out=mvex[:, 1, 0:1], in_=mv1[:, 0:1])
    nc.vector.tensor_mul(tcol[:, 1:2], mv1[:, 0:1], mv1[:, 0:1])
    nc.vector.tensor_add(mvex[:, 1, 1:2], tcol[:, 1:2], mv1[:, 1:2])
    # group sums over partitions: [16 groups, (mean0, ex20, mean1, ex21)]
    gsum = ps_o.tile([G // 2, 4], F32, name="gsum", tag="po")
    nc.tensor.matmul(
        gsum, lhsT=m1_sb, rhs=mvex.rearrange("p c k -> p (c k)"),
        start=True, stop=True,
    )
    warm(4)
    vals = small.tile([G // 2, 4], F32, name="vals")  # (rstd0, rstd1, m0, m1)
    gtmp = small.tile([G // 2, 4], F32, name="gtmp")
    gview = gsum.rearrange("g (c k) -> g c k", k=2)
    nc.vector.tensor_scalar_mul(
        vals.rearrange("g (c k) -> g c k", k=2)[:, 1, :], gview[:, :, 0],
        1.0 / CPG,
    )
    nc.vector.tensor_scalar_mul(gtmp[:, 0:2], gview[:, :, 1], 1.0 / CPG)
    nc.vector.tensor_mul(gtmp[:, 2:4], vals[:, 2:4], vals[:, 2:4])
    nc.vector.tensor_sub(gtmp[:, 0:2], gtmp[:, 0:2], gtmp[:, 2:4])
    # rstd = 1/sqrt(var + eps) (sqrt table preloaded; recip on DVE)
    nc.scalar.activation(gtmp[:, 2:4], gtmp[:, 0:2], Act.Sqrt, bias=eps16)
    nc.vector.reciprocal(vals[:, 0:2], gtmp[:, 2:4])
    # broadcast back to channels: [128, (rstd0, rstd1, m0, m1)]
    bc = ps_acc.tile([P, 4], F32, name="bc", tag="acc")
    nc.tensor.matmul(bc, lhsT=m2_sb, rhs=vals, start=True, stop=True)
    warm(4)
    # a = gamma * rstd; b = beta - mean * a; a8 = 8a (per channel cols)
    cols8 = small.tile([P, 8], F32, name="cols8")
    a_cols = [cols8[:, 0:1], cols8[:, 1:2]]
    b_cols = [cols8[:, 2:3], cols8[:, 3:4]]
    a8_cols = [cols8[:, 4:5], cols8[:, 5:6]]
    for ch in range(2):
        nc.vector.tensor_mul(a_cols[ch], bcols[:, ch, 4:5], bc[:, ch:ch + 1])
        nc.vector.tensor_mul(cols8[:, 6 + ch:7 + ch], bc[:, 2 + ch:3 + ch],
                             a_cols[ch])
        nc.vector.tensor_sub(b_cols[ch], bcols[:, ch, 5:6],
                             cols8[:, 6 + ch:7 + ch])
        nc.vector.tensor_scalar_mul(a8_cols[ch], a_cols[ch], WS)

    # scale wq/wk/wv rows by 8*a (per input channel) straight from the f32
    # staging into fp8 tiles. The x8 lift keeps the fp8 entries
    # (~N(0, a/16)) out of e4m3 subnormal range; the PSUM->SBUF casts
    # divide it back out.
    w_s = {}
    for wi, wname in ((0, "wqt"), (1, "wkt"), (2, "wvt")):
        ws = const.tile([P, 2, C], FP8, name=f"{wname}_s")
        for ci in range(2):
            nc.vector.tensor_scalar_mul(
                ws[:, ci, :], wstg[:, wi, ci, :], a8_cols[ci]
            )
        w_s[wname] = ws
    # wo8 = 8*wo in fp8 (no GroupNorm folding on the out-proj)
    wo8 = const.tile([P, 2, C], FP8)
    for ch in range(2):
        nc.scalar.mul(wo8[:, ch, :], wstg[:, 3, ch, :], WS)

    # projection bias columns: be = W b + bias (bf16 matvecs off the staged
    # weights; psum ring across three pools so the chain pipelines)
    b_bf = small.tile([P, 2], BF16, name="b_bf")
    for ch in range(2):
        nc.vector.tensor_copy(out=b_bf[:, ch:ch + 1], in_=b_cols[ch])
    _mvi = [0]
    mv_pools = [(ps_o, "po"), (ps_acc, "acc"), (ps, "mm")]

    def matvec_bias(wi, rhs_cols, bias_idx, out_name, out_dt=F32):
        outs = []
        for co in range(2):
            pool, tag = mv_pools[_mvi[0] % 3]
            _mvi[0] += 1
            pe = pool.tile([P, 1], F32, name="pe_mv", tag=tag)
            for ci in range(2):
                nc.tensor.matmul(
                    pe, lhsT=wstg[:, wi, ci, co * P:(co + 1) * P],
                    rhs=rhs_cols[ci], start=(ci == 0), stop=(ci == 1),
                )
            t = small.tile([P, 1], out_dt, name=f"{out_name}_{co}")
            nc.scalar.activation(
                t, pe, Act.Identity, bias=bcols[:, co, bias_idx:bias_idx + 1],
                scale=1.0,
            )
            outs.append(t)
        return outs

    bcol_list = [b_bf[:, 0:1], b_bf[:, 1:2]]
    be_q = matvec_bias(0, bcol_list, 0, "be_q")
    be_k = matvec_bias(1, bcol_list, 1, "be_k")
    vbv = matvec_bias(2, bcol_list, 2, "vbv", out_dt=BF16)
    bo_eff = matvec_bias(3, vbv, 3, "bo_eff")
    warm(4)
    # dummy exp: pulls the exp table load into the projection phase
    nc.scalar.activation(warm11, one11, Act.Exp, scale=1.0)

    # ---- projections (all DoubleRow fp8, contraction over full C=256) ----
    # K^T [C, N] fp8: psum[co,nb] = sum_ci wkt8[ci,co].T @ x8[ci, nb] (x8)
    # and q,k = psum/8 + bias. Casts alternate ACT / DVE (ACT is idle until
    # the attention loop's exp stream starts).
    k_sb = const.tile([P, 2, N], FP8)
    q_sb = const.tile([P, 2, NQ], FP8)
    IWS = 1.0 / WS
    # casts rotate over three engines (ACT/DVE/Pool) and the psum tiles over
    # three pools (5 in-flight banks) so the MM stream never waits on a cast;
    # two-engine casting was the pacer that kept HAM throttled at K=4.
    proj_ps = [(ps, "mm"), (ps_acc, "acc"), (ps, "mm"), (ps_o, "po"),
               (ps_acc, "acc")]
    _pi = [0]

    def proj_tile():
        pool, tag = proj_ps[_pi[0] % len(proj_ps)]
        _pi[0] += 1
        return pool.tile([P, QB], F32, name="pp", tag=tag)

    _ci = [0]

    def proj_cast(out, in_, bias):
        # only ACT/DVE can read PSUM; the 5-bank psum ring above keeps the
        # MM stream ~2.5 tiles ahead so the 2-engine cast pace (~325ns/tile)
        # doesn't stall the PE.
        i = _ci[0] % 2
        _ci[0] += 1
        if i == 0:
            nc.scalar.activation(out, in_, Act.Identity, bias=bias, scale=IWS)
        else:
            nc.vector.tensor_scalar(
                out=out, in0=in_, scalar1=IWS, scalar2=bias, op0=Alu.mult,
                op1=Alu.add,
            )

    for nb in range(NNB):
        for co in range(2):
            if nb < NQB:
                pq = proj_tile()
                nc.tensor.matmul(
                    pq, lhsT=w_s["wqt"][:, :, co * P:(co + 1) * P],
                    rhs=x8[:, :, nb * QB:(nb + 1) * QB],
                    start=True, stop=True, perf_mode=DR,
                )
                proj_cast(q_sb[:, co, nb * QB:(nb + 1) * QB], pq, be_q[co])
            pk = proj_tile()
            nc.tensor.matmul(
                pk, lhsT=w_s["wkt"][:, :, co * P:(co + 1) * P],
                rhs=x8[:, :, nb * QB:(nb + 1) * QB],
                start=True, stop=True, perf_mode=DR,
            )
            proj_cast(k_sb[:, co, nb * QB:(nb + 1) * QB], pk, be_k[co])

    # V [N, C] fp8 (bias folded into bo_eff): psum[nt] = x8_tile.T @ wvt8
    v_sb = const.tile([P, NKT, C], FP8)
    v_flat = v_sb.rearrange("p k c -> p (k c)")
    zero_col = const.tile([P, 1], F32)
    nc.vector.memset(zero_col, 0.0)
    for nt in range(0, NKT, 2):
        pv = proj_tile()
        for n2 in range(2):
            nc.tensor.matmul(
                pv[:, n2 * C:(n2 + 1) * C],
                lhsT=x8[:, :, (nt + n2) * P:(nt + n2 + 1) * P],
                rhs=w_s["wvt"][:, :, :],
                start=True, stop=True, perf_mode=DR,
            )
        proj_cast(v_flat[:, nt * C:(nt + 2) * C], pv, zero_col)

    # ---- attention, per query block; DoubleRow over key-tile pairs with a
    # batched exp (one ACT call per pair reading a 2-bank PSUM tile) and the
    # denominator as a DoubleRow ones-matmul on the PE. The softmax division
    # is commuted through the out-projection: out = (wo8 @ PV) * (1/(8*den))
    # + bo_eff + x, deferred one qb so nothing waits on the reciprocal.
    def epilogue_a(qb, dps, aps, at_sb, p_sb_dbg=None):
        # casts first: they release the PV accumulator banks immediately.
        # 1/8 keeps the heavy-tailed PV numerator inside fp8's +-240.
        nc.vector.tensor_scalar_mul(at_sb[:, 0, :], aps[0], 1.0 / 8.0)
        nc.vector.tensor_scalar_mul(at_sb[:, 1, :], aps[1], 1.0 / 8.0)
        den_r = work.tile([1, QB], F32, name="den_r", tag="den_r")
        nc.vector.reciprocal_approx_fast(out=den_r, in_=dps)
        den_b = work.tile([P, QB], F32, name="den_b", tag="den_b", bufs=2)
        nc.gpsimd.partition_broadcast(den_b, den_r)
        if DEBUG:
            nc.sync.dma_start(
                out=d["dbg_denr"][:, qb * QB:(qb + 1) * QB], in_=den_r
            )
            nc.sync.dma_start(
                out=d["dbg_at"][:, qb * 2 * QB:(qb + 1) * 2 * QB],
                in_=at_sb.rearrange("p h n -> p (h n)"),
            )
            if qb == 0:
                nc.sync.dma_start(
                    out=d["dbg_p"][:, :], in_=p_sb_dbg.rearrange("p k n -> p (k n)")
                )
        return den_b

    def epilogue_co(qb, at_sb, den_b, co, po_pool=None, po_tag="po"):
        po_pool = po_pool or ps_o
        po = po_pool.tile([P, QB], F32, name="po", tag=po_tag)
        nc.tensor.matmul(
            po, lhsT=wo8[:, :, co * P:(co + 1) * P], rhs=at_sb,
            start=True, stop=True, perf_mode=DR,
        )
        t1 = work.tile([P, QB], F32, name="t1", tag="t1")
        nc.vector.tensor_mul(t1, po, den_b)
        res = work.tile([P, QB], F32, name="res", tag="res", bufs=4)
        nc.vector.scalar_tensor_tensor(
            out=res, in0=t1, scalar=bo_eff[co],
            in1=x_sb[:, co, qb * QB:(qb + 1) * QB], op0=Alu.add, op1=Alu.add,
        )
        nc.sync.dma_start(
            out=out_d[co * P:(co + 1) * P, qb * QB:(qb + 1) * QB], in_=res
        )

    if DEBUG:
        nc.sync.dma_start(out=d["dbg_k"][:, :], in_=k_sb.rearrange("p h n -> p (h n)"))
        nc.sync.dma_start(out=d["dbg_q"][:, :], in_=q_sb.rearrange("p h n -> p (h n)"))
        nc.sync.dma_start(out=d["dbg_v"][:, :], in_=v_flat)

    pending = None
    pend_den = None
    for qb in range(NQB):
        p_sb = pblk.tile([P, NKT, QB], FP8, name="p_sb")
        dps = ps_d.tile([1, QB], F32, name="dps")
        aps = [
            ps_acc.tile([P, QB], F32, name="aps", tag="acc") for _ in range(2)
        ]
        at_sb = work.tile([P, 2, QB], FP8, name="at_sb", tag="at_sb", bufs=2)
        for j in range(NKP + 2):
            if j == 1 and pending is not None:
                pend_den = epilogue_a(*pending)
            if j == 2 and pending is not None:
                epilogue_co(pending[0], pending[3], pend_den, 0)
            if j == 3 and pending is not None:
                epilogue_co(pending[0], pending[3], pend_den, 1)
                pending = None
            if j < NKP:
                sps2 = ps.tile([P, 2 * QB], F32, name="sps2", tag="mm")
                for h2 in range(2):
                    kt = 2 * j + h2
                    nc.tensor.matmul(
                        sps2[:, h2 * QB:(h2 + 1) * QB],
                        lhsT=k_sb[:, :, kt * P:(kt + 1) * P],
                        rhs=q_sb[:, :, qb * QB:(qb + 1) * QB],
                        start=True, stop=True, perf_mode=DR,
                    )
                nc.scalar.activation(
                    p_sb[:, 2 * j:2 * j + 2, :], sps2, Act.Exp,
                    bias=esh_col, scale=SCALE,
                )
            if j >= 2:
                pj = j - 2
                pair = p_sb[:, 2 * pj:2 * pj + 2, :]
                for ch in range(2):
                    nc.tensor.matmul(
                        aps[ch],
                        lhsT=v_sb[:, 2 * pj:2 * pj + 2, ch * P:(ch + 1) * P],
                        rhs=pair,
                        start=(pj == 0), stop=(pj == NKP - 1),
                        perf_mode=DR, skip_group_check=True,
                    )
                nc.tensor.matmul(
                    dps, lhsT=ones8, rhs=pair,
                    start=(pj == 0), stop=(pj == NKP - 1),
                    perf_mode=DR, skip_group_check=True,
                )
        pending = (qb, dps, aps, at_sb, p_sb)
    pend_den = epilogue_a(*pending)
    epilogue_co(pending[0], pending[3], pend_den, 0, po_pool=ps, po_tag="mm")
    epilogue_co(pending[0], pending[3], pend_den, 1, po_pool=ps, po_tag="mm")

    for pool in (ps_o, ps_d, ps_acc, ps, work, pblk, small, stage, const):
        pool.release()


def build_program():
    global _NC
    if _NC is not None:
        return _NC
    nc = bacc.Bacc("TRN2", target_bir_lowering=False, debug=False,
                   num_devices=NCORES)
    d = {
        "x": nc.dram_tensor("x", [C, N], BF16, kind="ExternalInput"),
        "x8": nc.dram_tensor("x8", [C, N], FP8, kind="ExternalInput"),
        "wpack": nc.dram_tensor("wpack", [4 * C, C], BF16, kind="ExternalInput"),
        "bpack": nc.dram_tensor("bpack", [C, 6], F32, kind="ExternalInput"),
        "m1": nc.dram_tensor("m1", [P, G // 2], F32, kind="ExternalInput"),
        "m2": nc.dram_tensor("m2", [G // 2, P], F32, kind="ExternalInput"),
        "out": nc.dram_tensor("out", [C, NQ], F32, kind="ExternalOutput"),
    }
    if DEBUG:
        d.update({
            "dbg_k": nc.dram_tensor("dbg_k", [P, 2 * N], FP8, kind="ExternalOutput"),
            "dbg_q": nc.dram_tensor("dbg_q", [P, 2 * NQ], FP8, kind="ExternalOutput"),
            "dbg_v": nc.dram_tensor("dbg_v", [P, NKT * C], FP8, kind="ExternalOutput"),
            "dbg_p": nc.dram_tensor("dbg_p", [P, NKT * QB], FP8, kind="ExternalOutput"),
            "dbg_denr": nc.dram_tensor("dbg_denr", [1, NQ], F32, kind="ExternalOutput"),
            "dbg_at": nc.dram_tensor("dbg_at", [P, NQB * 2 * QB], FP8, kind="ExternalOutput"),
        })
    with tile.TileContext(nc) as tc:
        _body(tc, d)
    nc.compile()
    _NC = nc
    return nc


def make_in_maps(x, gamma, beta, wq, bq, wk, bk, wv, bv, wo, bo):
    import ml_dtypes

    f32c = lambda a: np.ascontiguousarray(np.asarray(a, dtype=np.float32))
    x = f32c(x)
    wpack = np.concatenate(
        [f32c(np.asarray(w, np.float32).T) for w in (wq, wk, wv, wo)], axis=0
    )
    bpack = np.stack(
        [f32c(v).reshape(C) for v in (bq, bk, bv, bo, gamma, beta)], axis=1
    )
    m1 = np.zeros((P, G // 2), np.float32)
    for g in range(G // 2):
        m1[8 * g:8 * g + 8, g] = 1.0
    base = {
        "wpack": np.ascontiguousarray(wpack.astype(ml_dtypes.bfloat16)),
        "bpack": np.ascontiguousarray(bpack),
        "m1": m1,
        "m2": np.ascontiguousarray(m1.T),
    }
    import ml_dtypes

    in_maps = []
    for core in range(NCORES):
        b, h = divmod(core, 2)
        xb = x[b].reshape(C, N)
        if h:
            xb = np.concatenate([xb[:, NQ:], xb[:, :NQ]], axis=1)
        in_maps.append({
            **base,
            "x": np.ascontiguousarray(xb.astype(ml_dtypes.bfloat16)),
            "x8": np.ascontiguousarray(xb.astype(ml_dtypes.float8_e4m3)),
        })
    return in_maps


def kernel(x, gamma, beta, wq, bq, wk, bk, wv, bv, wo, bo):
    global LAST_RESULTS
    from concourse.bass_utils import run_bass_kernel_spmd

    nc = build_program()
    in_maps = make_in_maps(x, gamma, beta, wq, bq, wk, bk, wv, bv, wo, bo)
    res = run_bass_kernel_spmd(nc, in_maps, core_ids=list(range(NCORES)))
    LAST_RESULTS = res
    out = np.empty((B, C, N), np.float32)
    for core in range(NCORES):
        b, h = divmod(core, 2)
        out[b][:, h * NQ:(h + 1) * NQ] = res.results[core]["out"]
    return out.reshape(B, C, H, W)


# revision 36
# speedup vs baseline: 1.1138x; 1.0329x over previous
"""AttnBlock (GroupNorm + single-head self-attention + residual) on 8 TRN2 cores.

Sharding: data-parallel over (batch b, query-half h) -> 8 shards. Each core
receives the full [C, N] image of its batch (columns rolled so that its own
query half always occupies columns 0:NQ), computes GroupNorm stats + K/V over
the whole image, Q over its half, and a flash-style attention in which scores
are produced directly transposed (S^T = K^T.T @ Q^T tiles) so softmax
normalization is done via a ones-vector matmul and no PE transposes of P are
needed.

All large matmuls (projections, S^T, PV, denominator, out-proj) run as fp8e4
DoubleRow matmuls: lhsT [128, 2, M] / rhs [128, 2, N] contract 256 deep in a
single instruction at ~2x bf16 FLOP rate. Weights are scaled x8 before the
fp8 cast (their entries are ~N(0, 1/16) and would hit e4m3 subnormals); the
scale is removed in the PSUM->SBUF cast. The softmax exp is shifted by -2
(exp(s/16 - 2)) so P fits e4m3's +-240 range; numerator and denominator share
the shift so the ratio is unchanged. exp runs on ACT in 2-key-tile batches
([128,1024] over a 2-bank PSUM tile) to amortize the per-call overhead, and
the denominator is a DoubleRow ones-matmul on the PE (accumulated in a
[1,512] PSUM bank), keeping the DVE free for casts and the epilogue.
"""

import os
import sys

import numpy as np

for _p in ("/opt/trn_rl_repo", "/root/.axon_site/_ro/trn_rl_repo"):
    if os.path.isdir(_p) and _p not in sys.path:
        sys.path.insert(0, _p)

import concourse.bass as bass  # noqa: E402
import concourse.tile as tile  # noqa: E402
from concourse import bacc, mybir  # noqa: E402
from concourse.masks import make_identity  # noqa: E402

# The agent image's antenv lacks axon_hooks; if BASS_TRACE is set in the
# environment, run_bass_kernel_spmd would crash importing it. Provide a stub
# (profiling degrades gracefully to "hook isn't registered").
try:
    import antenv.axon_hooks  # noqa: F401
except ImportError:
    import types as _types

    _m = _types.ModuleType("antenv.axon_hooks")
    _h = [None]
    _m.set_axon_ntff_profile_hook = lambda h: _h.__setitem__(0, h)
    _m.get_axon_ntff_profile_hook = lambda: _h[0]
    sys.modules["antenv.axon_hooks"] = _m

B, C, H, W = 4, 256, 64, 64
N = H * W  # 4096 pixels
NQ = N // 2  # 2048 queries per core
G = 32  # groups
CPG = C // G  # 8 channels per group
EPS = 1e-5
NCORES = 8
SCALE = float(C) ** -0.5  # 0.0625
ESHIFT = -3.0  # exp(s*SCALE + ESHIFT): data max logit 7.95 -> P <= ~141 < 240
WS = 8.0  # weight fp8 pre-scale (entries ~N(0,1/16) need lifting)

F32 = mybir.dt.float32
BF16 = mybir.dt.bfloat16
FP8 = mybir.dt.float8e4

QB = 512  # query block (free dim of S^T / PV matmuls)
NQB = NQ // QB  # 4 query blocks
NKT = N // 128  # 32 key tiles
NKP = NKT // 2  # 16 key-tile pairs (DoubleRow granularity)
NNB = N // QB  # 8 pixel blocks for K/V projections
P = 128

DEBUG = bool(int(os.environ.get("KDEBUG", "0")))

Act = mybir.ActivationFunctionType
Alu = mybir.AluOpType
Axis = mybir.AxisListType
DR = mybir.MatmulPerfMode.DoubleRow

_NC = None
LAST_RESULTS = None


def _body(tc, d):
    nc = tc.nc
    x_d = d["x"]
    out_d = d["out"]

    const = tc.alloc_tile_pool(name="const", bufs=1)
    stage = tc.alloc_tile_pool(name="stage", bufs=2)
    small = tc.alloc_tile_pool(name="small", bufs=1)
    pblk = tc.alloc_tile_pool(name="pblk", bufs=2)
    work = tc.alloc_tile_pool(name="work", bufs=2)
    # PSUM budget (8 banks): S-pair tiles 2x[P,1024] = 4, aps 2x[P,512] = 2,
    # dps [1,512] = 1, po [P,512] = 1.
    ps = tc.alloc_tile_pool(name="ps", bufs=2, space="PSUM")
    ps_acc = tc.alloc_tile_pool(name="ps_acc", bufs=2, space="PSUM")
    ps_d = tc.alloc_tile_pool(name="ps_d", bufs=1, space="PSUM")
    ps_o = tc.alloc_tile_pool(name="ps_o", bufs=1, space="PSUM")

    # ---- x in SBUF first: bf16 [128, 2(ch), 4096] (residual + stats) and
    # fp8 [128, 2, 4096] (matmul operand), both cast on host; chunked so
    # bn_stats overlaps the transfer ----
    x_sb = const.tile([P, 2, N], BF16)
    x8 = const.tile([P, 2, N], FP8)
    x_src = x_d.ap().rearrange("(h p) n -> p h n", p=P)
    x8_src = d["x8"].ap().rearrange("(h p) n -> p h n", p=P)

    # PE warm-up: keep the HAM activity monitor busy during the DMA/stats
    # window so projections and attention run at full clock from the start.
    # Memsets go FIRST so the warm matmuls aren't queued behind the bn_stats
    # chain in DVE program order (PE would idle for the whole DMA phase).
    wu_w = const.tile([P, P], BF16)
    nc.vector.memset(wu_w, 0.0)
    wu_x = const.tile([P, 2 * P], BF16)
    nc.vector.memset(wu_x, 0.0)
    wu_ps = ps.tile([P, QB], F32, name="wu_ps", tag="mm")

    def warm(n):
        for _ in range(n):
            nc.tensor.matmul(
                wu_ps[:, 0:2 * P], lhsT=wu_w, rhs=wu_x, start=True, stop=True
            )

    warm(2)

    # ---- head DMA: per-queue DMA bandwidth is only ~90GB/s, so the hot x8
    # (1MB, feeds stats AND all matmuls) goes FIRST: one chunk on sync, one
    # on ACT, two on gpsimd. Everything else (masks, biases, weight pack,
    # and the bf16 residual x) queues up behind it.
    x8q = [nc.sync, nc.gpsimd, nc.scalar, nc.gpsimd]
    for c in range(4):
        sl = (slice(None), slice(None), slice(c * 2 * QB, (c + 1) * 2 * QB))
        x8q[c].dma_start(out=x8[sl], in_=x8_src[sl])
    m1_sb = const.tile([P, G // 2], F32)
    nc.sync.dma_start(out=m1_sb, in_=d["m1"][:, :])
    m2_sb = const.tile([G // 2, P], F32)
    nc.sync.dma_start(out=m2_sb, in_=d["m2"][:, :])
    # packed biases/affine: rows (bq,bk,bv,bo,gamma,beta) -> per-channel cols
    bcols = const.tile([P, 2, 6], F32)
    nc.sync.dma_start(
        out=bcols, in_=d["bpack"].ap().rearrange("(h p) s -> p h s", p=P)
    )
    # all four weight matrices (bf16) in one DMA, behind x8 c1 on gpsimd
    wstg = stage.tile([P, 4, 2, C], BF16, name="wstg", tag="wstg")
    nc.gpsimd.dma_start(
        out=wstg, in_=d["wpack"].ap().rearrange("(w h p) co -> p w h co", p=P, h=2)
    )
    xsbq = [nc.sync, nc.gpsimd, nc.scalar, nc.scalar]
    for c in range(4):
        sl = (slice(None), slice(None), slice(c * 2 * QB, (c + 1) * 2 * QB))
        xsbq[c].dma_start(out=x_sb[sl], in_=x_src[sl])

    # GroupNorm stats from the fp8 x (self-consistent: the projections
    # consume the same fp8 values; adds ~0.1% to the error budget). All on
    # DVE bn_stats: ACT must stay quiet so its sqrt table survives to the
    # dance (Identity/Square passes were thrashing the table sets).
    bn_st0 = small.tile([P, NNB, 6], F32, name="bn_st0")
    bn_st1 = small.tile([P, NNB, 6], F32, name="bn_st1")
    for c in range(4):
        for k in range(2):
            j = 2 * c + k
            cols = slice(j * QB, (j + 1) * QB)
            nc.vector.bn_stats(out=bn_st0[:, j, :], in_=x8[:, 0, cols])
            nc.vector.bn_stats(out=bn_st1[:, j, :], in_=x8[:, 1, cols])
        warm(10)

    # ---- constants ----
    one11 = const.tile([1, 1], F32)
    nc.vector.memset(one11, 1.0)
    # padded to 16B so the DoubleRow k-tile stride meets walrus' step%16==0.
    # Value 1.0 balances the at_sb 1/8 down-scale and the x8 wo lift:
    # po*den_r = (8/8)*wo@at_un / den = wo@at_un/den.
    ones8_pad = const.tile([P, 2, 16], FP8)
    nc.vector.memset(ones8_pad, 1.0)
    ones8 = ones8_pad[:, :, 0:1]
    eps16 = const.tile([G // 2, 1], F32)
    nc.vector.memset(eps16, EPS)
    esh_col = const.tile([P, 1], F32)
    nc.vector.memset(esh_col, ESHIFT)
    # preload the ACT sqrt table; the exp set is loaded by a dummy exp right
    # after the dance so the switch overlaps the projection phase.
    warm11 = small.tile([1, 1], F32)
    nc.scalar.activation(warm11, one11, Act.Sqrt, scale=1.0)

    # ---- GroupNorm dance: per-channel (mean, E[x^2]) -> group reduce via
    # mask matmul -> rstd -> broadcast back via mask matmul -> a/b columns.
    mvex = small.tile([P, 2, 2], F32, name="mvex")  # (mean, ex2) per ch-half
    mv0 = small.tile([P, 2], F32, name="mv0")
    nc.vector.bn_aggr(out=mv0, in_=bn_st0)
    mv1 = small.tile([P, 2], F32, name="mv1")
    nc.vector.bn_aggr(out=mv1, in_=bn_st1)
    tcol = small.tile([P, 6], F32, name="tcol")
    # ch0: mean, ex2 = var + mean^2
    nc.vector.tensor_copy(out=mvex[:, 0, 0:1], in_=mv0[:, 0:1])
    nc.vector.tensor_mul(tcol[:, 0:1], mv0[:, 0:1], mv0[:, 0:1])
    nc.vector.tensor_add(mvex[:, 0, 1:2], tcol[:, 0:1], mv0[:, 1:2])
    # ch1: mean, ex2 = var + mean^2
    nc.vector.tensor_copy(out=mvex[:, 1, 0:1], in_=mv1[:, 0:1])
    nc.vector.tensor_mul(tcol[:, 1:2], mv1[:, 0:1], mv1[:, 0:1])
    nc.vector.tensor_add(mvex[:, 1, 1:2], tcol[:, 1:2], mv1[:, 1:2])
    # group sums over partitions: [16 groups, (mean0, ex20, mean1, ex21)]
    gsum = ps_o.tile([G // 2, 4], F32, name="gsum", tag="po")
    nc.tensor.matmul(
        gsum, lhsT=m1_sb, rhs=mvex.rearrange("p c k -> p (c k)"),
        start=True, stop=True,
    )
    warm(4)
    vals = small.tile([G // 2, 4], F32, name="vals")  # (rstd0, rstd1, m0, m1)
    gtmp = small.tile([G // 2, 4], F32, name="gtmp")
    gview = gsum.rearrange("g (c k) -> g c k", k=2)
    nc.vector.tensor_scalar_mul(
        vals.rearrange("g (c k) -> g c k", k=2)[:, 1, :], gview[:, :, 0],
        1.0 / CPG,
    )
    nc.vector.tensor_scalar_mul(gtmp[:, 0:2], gview[:, :, 1], 1.0 / CPG)
    nc.vector.tensor_mul(gtmp[:, 2:4], vals[:, 2:4], vals[:, 2:4])
    nc.vector.tensor_sub(gtmp[:, 0:2], gtmp[:, 0:2], gtmp[:, 2:4])
    # rstd = 1/sqrt(var + eps) (sqrt table preloaded; recip on DVE)
    nc.scalar.activation(gtmp[:, 2:4], gtmp[:, 0:2], Act.Sqrt, bias=eps16)
    nc.vector.reciprocal(vals[:, 0:2], gtmp[:, 2:4])
    # broadcast back to channels: [128, (rstd0, rstd1, m0, m1)]
    bc = ps_acc.tile([P, 4], F32, name="bc", tag="acc")
    nc.tensor.matmul(bc, lhsT=m2_sb, rhs=vals, start=True, stop=True)
    warm(4)
    # a = gamma * rstd; b = beta - mean * a; a8 = 8a (per channel cols)
    cols8 = small.tile([P, 8], F32, name="cols8")
    a_cols = [cols8[:, 0:1], cols8[:, 1:2]]
    b_cols = [cols8[:, 2:3], cols8[:, 3:4]]
    a8_cols = [cols8[:, 4:5], cols8[:, 5:6]]
    for ch in range(2):
        nc.vector.tensor_mul(a_cols[ch], bcols[:, ch, 4:5], bc[:, ch:ch + 1])
        nc.vector.tensor_mul(cols8[:, 6 + ch:7 + ch], bc[:, 2 + ch:3 + ch],
                             a_cols[ch])
        nc.vector.tensor_sub(b_cols[ch], bcols[:, ch, 5:6],
                             cols8[:, 6 + ch:7 + ch])
        nc.vector.tensor_scalar_mul(a8_cols[ch], a_cols[ch], WS)

    # scale wq/wk/wv rows by 8*a (per input channel) straight from the f32
    # staging into fp8 tiles. The x8 lift keeps the fp8 entries
    # (~N(0, a/16)) out of e4m3 subnormal range; the PSUM->SBUF casts
    # divide it back out.
    w_s = {}
    for wi, wname in ((0, "wqt"), (1, "wkt"), (2, "wvt")):
        ws = const.tile([P, 2, C], FP8, name=f"{wname}_s")
        for ci in range(2):
            nc.vector.tensor_scalar_mul(
                ws[:, ci, :], wstg[:, wi, ci, :], a8_cols[ci]
            )
        w_s[wname] = ws
    # wo8 = 8*wo in fp8 (no GroupNorm folding on the out-proj)
    wo8 = const.tile([P, 2, C], FP8)
    for ch in range(2):
        nc.scalar.mul(wo8[:, ch, :], wstg[:, 3, ch, :], WS)

    # projection bias columns: be = W b + bias (bf16 matvecs off the staged
    # weights; psum ring across three pools so the chain pipelines)
    b_bf = small.tile([P, 2], BF16, name="b_bf")
    for ch in range(2):
        nc.vector.tensor_copy(out=b_bf[:, ch:ch + 1], in_=b_cols[ch])
    _mvi = [0]
    mv_pools = [(ps_o, "po"), (ps_acc, "acc"), (ps, "mm")]

    def matvec_bias(wi, rhs_cols, bias_idx, out_name, out_dt=F32):
        outs = []
        for co in range(2):
            pool, tag = mv_pools[_mvi[0] % 3]
            _mvi[0] += 1
            pe = pool.tile([P, 1], F32, name="pe_mv", tag=tag)
            for ci in range(2):
                nc.tensor.matmul(
                    pe, lhsT=wstg[:, wi, ci, co * P:(co + 1) * P],
                    rhs=rhs_cols[ci], start=(ci == 0), stop=(ci == 1),
                )
            t = small.tile([P, 1], out_dt, name=f"{out_name}_{co}")
            nc.scalar.activation(
                t, pe, Act.Identity, bias=bcols[:, co, bias_idx:bias_idx + 1],
                scale=1.0,
            )
            outs.append(t)
        return outs

    bcol_list = [b_bf[:, 0:1], b_bf[:, 1:2]]
    be_q = matvec_bias(0, bcol_list, 0, "be_q")
    be_k = matvec_bias(1, bcol_list, 1, "be_k")
    vbv = matvec_bias(2, bcol_list, 2, "vbv", out_dt=BF16)
    bo_eff = matvec_bias(3, vbv, 3, "bo_eff")
    warm(4)
    # dummy exp: pulls the exp table load into the projection phase
    nc.scalar.activation(warm11, one11, Act.Exp, scale=1.0)

    # ---- projections (all DoubleRow fp8, contraction over full C=256) ----
    # K^T [C, N] fp8: psum[co,nb] = sum_ci wkt8[ci,co].T @ x8[ci, nb] (x8)
    # and q,k = psum/8 + bias. Casts alternate ACT / DVE (ACT is idle until
    # the attention loop's exp stream starts).
    k_sb = const.tile([P, 2, N], FP8)
    q_sb = const.tile([P, 2, NQ], FP8)
    IWS = 1.0 / WS
    # casts rotate over three engines (ACT/DVE/Pool) and the psum tiles over
    # three pools (5 in-flight banks) so the MM stream never waits on a cast;
    # two-engine casting was the pacer that kept HAM throttled at K=4.
    proj_ps = [(ps, "mm"), (ps_acc, "acc"), (ps, "mm"), (ps_o, "po"),
               (ps_acc, "acc")]
    _pi = [0]

    def proj_tile():
        pool, tag = proj_ps[_pi[0] % len(proj_ps)]
        _pi[0] += 1
        return pool.tile([P, QB], F32, name="pp", tag=tag)

    _ci = [0]

    def proj_cast(out, in_, bias):
        # only ACT/DVE can read PSUM; the 5-bank psum ring above keeps the
        # MM stream ~2.5 tiles ahead so the 2-engine cast pace (~325ns/tile)
        # doesn't stall the PE.
        i = _ci[0] % 2
        _ci[0] += 1
        if i == 0:
            nc.scalar.activation(out, in_, Act.Identity, bias=bias, scale=IWS)
        else:
            nc.vector.tensor_scalar(
                out=out, in0=in_, scalar1=IWS, scalar2=bias, op0=Alu.mult,
                op1=Alu.add,
            )

    for nb in range(NNB):
        for co in range(2):
            if nb < NQB:
                pq = proj_tile()
                nc.tensor.matmul(
                    pq, lhsT=w_s["wqt"][:, :, co * P:(co + 1) * P],
                    rhs=x8[:, :, nb * QB:(nb + 1) * QB],
                    start=True, stop=True, perf_mode=DR,
                )
                proj_cast(q_sb[:, co, nb * QB:(nb + 1) * QB], pq, be_q[co])
            pk = proj_tile()
            nc.tensor.matmul(
                pk, lhsT=w_s["wkt"][:, :, co * P:(co + 1) * P],
                rhs=x8[:, :, nb * QB:(nb + 1) * QB],
                start=True, stop=True, perf_mode=DR,
            )
            proj_cast(k_sb[:, co, nb * QB:(nb + 1) * QB], pk, be_k[co])

    # V [N, C] fp8 (bias folded into bo_eff): psum[nt] = x8_tile.T @ wvt8
    v_sb = const.tile([P, NKT, C], FP8)
    v_flat = v_sb.rearrange("p k c -> p (k c)")
    zero_col = const.tile([P, 1], F32)
    nc.vector.memset(zero_col, 0.0)
    for nt in range(0, NKT, 2):
        pv = proj_tile()
        for n2 in range(2):
            nc.tensor.matmul(
                pv[:, n2 * C:(n2 + 1) * C],
                lhsT=x8[:, :, (nt + n2) * P:(nt + n2 + 1) * P],
                rhs=w_s["wvt"][:, :, :],
                start=True, stop=True, perf_mode=DR,
            )
        proj_cast(v_flat[:, nt * C:(nt + 2) * C], pv, zero_col)

    # ---- attention, per query block; DoubleRow over key-tile pairs with a
    # batched exp (one ACT call per pair reading a 2-bank PSUM tile) and the
    # denominator as a DoubleRow ones-matmul on the PE. The softmax division
    # is commuted through the out-projection: out = (wo8 @ PV) * (1/(8*den))
    # + bo_eff + x, deferred one qb so nothing waits on the reciprocal.
    def epilogue_a(qb, dps, aps, at_sb, p_sb_dbg=None):
        # casts first: they release the PV accumulator banks immediately.
        # 1/8 keeps the heavy-tailed PV numerator inside fp8's +-240.
        nc.vector.tensor_scalar_mul(at_sb[:, 0, :], aps[0], 1.0 / 8.0)
        nc.vector.tensor_scalar_mul(at_sb[:, 1, :], aps[1], 1.0 / 8.0)
        den_r = work.tile([1, QB], F32, name="den_r", tag="den_r")
        nc.vector.reciprocal_approx_fast(out=den_r, in_=dps)
        den_b = work.tile([P, QB], F32, name="den_b", tag="den_b", bufs=2)
        nc.gpsimd.partition_broadcast(den_b, den_r)
        if DEBUG:
            nc.sync.dma_start(
                out=d["dbg_denr"][:, qb * QB:(qb + 1) * QB], in_=den_r
            )
            nc.sync.dma_start(
                out=d["dbg_at"][:, qb * 2 * QB:(qb + 1) * 2 * QB],
                in_=at_sb.rearrange("p h n -> p (h n)"),
            )
            if qb == 0:
                nc.sync.dma_start(
                    out=d["dbg_p"][:, :], in_=p_sb_dbg.rearrange("p k n -> p (k n)")
                )
        return den_b

    def epilogue_co(qb, at_sb, den_b, co, po_pool=None, po_tag="po"):
        po_pool = po_pool or ps_o
        po = po_pool.tile([P, QB], F32, name="po", tag=po_tag)
        nc.tensor.matmul(
            po, lhsT=wo8[:, :, co * P:(co + 1) * P], rhs=at_sb,
            start=True, stop=True, perf_mode=DR,
        )
        t1 = work.tile([P, QB], F32, name="t1", tag="t1")
        nc.vector.tensor_mul(t1, po, den_b)
        res = work.tile([P, QB], F32, name="res", tag="res", bufs=4)
        nc.vector.scalar_tensor_tensor(
            out=res, in0=t1, scalar=bo_eff[co],
            in1=x_sb[:, co, qb * QB:(qb + 1) * QB], op0=Alu.add, op1=Alu.add,
        )
        nc.sync.dma_start(
            out=out_d[co * P:(co + 1) * P, qb * QB:(qb + 1) * QB], in_=res
        )

    if DEBUG:
        nc.sync.dma_start(out=d["dbg_k"][:, :], in_=k_sb.rearrange("p h n -> p (h n)"))
        nc.sync.dma_start(out=d["dbg_q"][:, :], in_=q_sb.rearrange("p h n -> p (h n)"))
        nc.sync.dma_start(out=d["dbg_v"][:, :], in_=v_flat)

    pending = None
    pend_den = None
    for qb in range(NQB):
        p_sb = pblk.tile([P, NKT, QB], FP8, name="p_sb")
        dps = ps_d.tile([1, QB], F32, name="dps")
        aps = [
            ps_acc.tile([P, QB], F32, name="aps", tag="acc") for _ in range(2)
        ]
        at_sb = work.tile([P, 2, QB], FP8, name="at_sb", tag="at_sb", bufs=2)
        for j in range(NKP + 2):
            if j == 1 and pending is not None:
                pend_den = epilogue_a(*pending)
            if j == 2 and pending is not None:
                epilogue_co(pending[0], pending[3], pend_den, 0)
            if j == 3 and pending is not None:
                epilogue_co(pending[0], pending[3], pend_den, 1)
                pending = None
            if j < NKP:
                sps2 = ps.tile([P, 2 * QB], F32, name="sps2", tag="mm")
                for h2 in range(2):
                    kt = 2 * j + h2
                    nc.tensor.matmul(
                        sps2[:, h2 * QB:(h2 + 1) * QB],
                        lhsT=k_sb[:, :, kt * P:(kt + 1) * P],
                        rhs=q_sb[:, :, qb * QB:(qb + 1) * QB],
                        start=True, stop=True, perf_mode=DR,
                    )
                nc.scalar.activation(
                    p_sb[:, 2 * j:2 * j + 2, :], sps2, Act.Exp,
                    bias=esh_col, scale=SCALE,
                )
            if j >= 2:
                pj = j - 2
                pair = p_sb[:, 2 * pj:2 * pj + 2, :]
                for ch in range(2):
                    nc.tensor.matmul(
                        aps[ch],
                        lhsT=v_sb[:, 2 * pj:2 * pj + 2, ch * P:(ch + 1) * P],
                        rhs=pair,
                        start=(pj == 0), stop=(pj == NKP - 1),
                        perf_mode=DR, skip_group_check=True,
                    )
                nc.tensor.matmul(
                    dps, lhsT=ones8, rhs=pair,
                    start=(pj == 0), stop=(pj == NKP - 1),
                    perf_mode=DR, skip_group_check=True,
                )
        pending = (qb, dps, aps, at_sb, p_sb)
    pend_den = epilogue_a(*pending)
    epilogue_co(pending[0], pending[3], pend_den, 0, po_pool=ps, po_tag="mm")
    epilogue_co(pending[0], pending[3], pend_den, 1, po_pool=ps, po_tag="mm")

    for pool in (ps_o, ps_d, ps_acc, ps, work, pblk, small, stage, const):
        pool.release()


def build_program():
    global _NC
    if _NC is not None:
        return _NC
    nc = bacc.Bacc("TRN2", target_bir_lowering=False, debug=False,
                   num_devices=NCORES)
    d = {
        "x": nc.dram_tensor("x", [C, N], BF16, kind="ExternalInput"),
        "x8": nc.dram_tensor("x8", [C, N], FP8, kind="ExternalInput"),
        "wpack": nc.dram_tensor("wpack", [4 * C, C], BF16, kind="ExternalInput"),
        "bpack": nc.dram_tensor("bpack", [C, 6], F32, kind="ExternalInput"),
        "m1": nc.dram_tensor("m1", [P, G // 2], F32, kind="ExternalInput"),
        "m2": nc.dram_tensor("m2", [G // 2, P], F32, kind="ExternalInput"),
        "out": nc.dram_tensor("out", [C, NQ], F32, kind="ExternalOutput"),
    }
    if DEBUG:
        d.update({
            "dbg_k": nc.dram_tensor("dbg_k", [P, 2 * N], FP8, kind="ExternalOutput"),
            "dbg_q": nc.dram_tensor("dbg_q", [P, 2 * NQ], FP8, kind="ExternalOutput"),
            "dbg_v": nc.dram_tensor("dbg_v", [P, NKT * C], FP8, kind="ExternalOutput"),
            "dbg_p": nc.dram_tensor("dbg_p", [P, NKT * QB], FP8, kind="ExternalOutput"),
            "dbg_denr": nc.dram_tensor("dbg_denr", [1, NQ], F32, kind="ExternalOutput"),
            "dbg_at": nc.dram_tensor("dbg_at", [P, NQB * 2 * QB], FP8, kind="ExternalOutput"),
        })
    with tile.TileContext(nc) as tc:
        _body(tc, d)
    nc.compile()
    _NC = nc
    return nc


def make_in_maps(x, gamma, beta, wq, bq, wk, bk, wv, bv, wo, bo):
    import ml_dtypes

    f32c = lambda a: np.ascontiguousarray(np.asarray(a, dtype=np.float32))
    x = f32c(x)
    wpack = np.concatenate(
        [f32c(np.asarray(w, np.float32).T) for w in (wq, wk, wv, wo)], axis=0
    )
    bpack = np.stack(
        [f32c(v).reshape(C) for v in (bq, bk, bv, bo, gamma, beta)], axis=1
    )
    m1 = np.zeros((P, G // 2), np.float32)
    for g in range(G // 2):
        m1[8 * g:8 * g + 8, g] = 1.0
    base = {
        "wpack": np.ascontiguousarray(wpack.astype(ml_dtypes.bfloat16)),
        "bpack": np.ascontiguousarray(bpack),
        "m1": m1,
        "m2": np.ascontiguousarray(m1.T),
    }
    import ml_dtypes

    in_maps = []
    for core in range(NCORES):
        b, h = divmod(core, 2)
        xb = x[b].reshape(C, N)
        if h:
            xb = np.concatenate([xb[:, NQ:], xb[:, :NQ]], axis=1)
        in_maps.append({
            **base,
            "x": np.ascontiguousarray(xb.astype(ml_dtypes.bfloat16)),
            "x8": np.ascontiguousarray(xb.astype(ml_dtypes.float8_e4m3)),
        })
    return in_maps


def kernel(x, gamma, beta, wq, bq, wk, bk, wv, bv, wo, bo):
    global LAST_RESULTS
    from concourse.bass_utils import run_bass_kernel_spmd

    nc = build_program()
    in_maps = make_in_maps(x, gamma, beta, wq, bq, wk, bk, wv, bv, wo, bo)
    res = run_bass_kernel_spmd(nc, in_maps, core_ids=list(range(NCORES)))
    LAST_RESULTS = res
    out = np.empty((B, C, N), np.float32)
    for core in range(NCORES):
        b, h = divmod(core, 2)
        out[b][:, h * NQ:(h + 1) * NQ] = res.results[core]["out"]
    return out.reshape(B, C, H, W)
